# revision 30
# baseline (speedup 1.0000x reference)
"""Trainium2 Bass kernel for dilated sliding-window attention (AttnWrapper).

Reference computation (all fp32):
  combined = [begin | main | end]                       # [8256, 768]
  keys[t]  = combined[t + 32 + off], off in +-{4..32}   # 16 dilated window keys
  q = (main @ wq.T + bq) * 96**-0.5
  k/v = keys @ w{k,v}.T + b{k,v}
  attn = softmax(q.k), ctx = attn.v, out = [main | ctx @ wo.T + bo]

Sharding: tokens across 8 cores (1024 each) with a 64-row halo of the
combined buffer; weights replicated. Each core computes attn_outT
[768, 1024]; the host transposes and concatenates with main.

Device-side notes (v2 — all-bf16 matmul pipeline):
 - All matmul operands bf16 (cast on host). bf16 runs 1 cycle/col at any
   size on PE (f32r needs >=256 cols), so score matmuls stream only each
   key-chunk's valid token window (128/192/192/192/64) instead of 512.
 - bk dropped (softmax shift invariance); bv folded into bo2 = wo@bv + bo;
   q scale and bq folded into wq/bq on the host.
 - A ones-column per V head block makes the ctx matmul also produce the
   softmax denominator (row 96 of the ctx PSUM tile). 1/d runs on DVE
   (reciprocal_approx_fast — keeps ACT on the Exp table, avoiding
   ~1.3us table reloads per Ln/Exp switch), broadcast on GPSIMD,
   multiply on DVE; all off the PE critical path.
 - Phases: v-proj, k-proj, then q-proj software-pipelined with group-0
   attention (head h's attention is emitted during head h+1's q chains),
   then group-1 attention interleaved with group-0 out-projection.
 - Per-chunk input DMAs ordered so the first v matmul starts ~8us in.
"""

import numpy as np

EMBED_DIM = 768
NUM_HEADS = 8
HEAD_DIM = 96
OVERLAP = 32
HALO = 2 * OVERLAP          # 64 extra combined rows per core
N_LINES = 8192
N_CORES = 8
TOK = N_LINES // N_CORES    # 1024 tokens per core
ROWS = TOK + HALO           # 1088 combined rows per core
GRP = 512                   # tokens per attention group
NG = TOK // GRP             # 2 groups
# key chunks (start, end), valid token windows (w0, w1), mask index
CHUNKS = [(0, 128, 0, 128, 0), (128, 256, 64, 256, 1), (256, 384, 192, 384, 1),
          (384, 512, 320, 512, 1), (512, 576, 448, 512, 2)]
VBLK = HEAD_DIM + 1         # 97: v head block + ones column
KC = EMBED_DIM // 128       # 6 contraction chunks of 128
NVC = (ROWS + 127) // 128   # 9 v row-chunks (8x128 + 64)
NCONST = 8 + 6 + 192        # bq | bo2 | masks(bf16 pairs)


def _ctx_pieces():
    """Per-head splits of the 96 ctx rows into segments legal on both the
    per-head source side (base = row) and the packed-768 destination side
    (base = global row % 128), given the partition-access rule: base must
    be 0/32/64/96 and the range must not cross the next alignment level
    (0->128, 32->32, 64->64, 96->32 max sizes)."""
    def legal(s, size):
        return size <= {0: 128, 32: 32, 64: 64, 96: 32}.get(s, -1)

    out = []
    for h in range(NUM_HEADS):
        g0 = HEAD_DIM * h
        merged = []
        for a in range(g0, g0 + HEAD_DIM, 32):
            if merged:
                m0, m1 = merged[-1]
                if (m0 // 128 == (a + 31) // 128
                        and legal(m0 % 128, a + 32 - m0)
                        and legal(m0 - g0, a + 32 - m0)):
                    merged[-1] = (m0, a + 32)
                    continue
            merged.append((a, a + 32))
        out.append([(a - g0, b - g0, a // 256, (a // 128) % 2, a % 128)
                    for a, b in merged])
    return out


CTX_PIECES = _ctx_pieces()


def _build_program():
    import concourse.bacc as bacc
    import concourse.mybir as mybir
    from concourse.tile import TileContext

    f32 = mybir.dt.float32
    bf16 = mybir.dt.bfloat16
    D = EMBED_DIM

    nc = bacc.Bacc("TRN2", target_bir_lowering=False, debug=False,
                   enable_asserts=False, num_devices=N_CORES)

    fp8 = mybir.dt.float8e4
    PM = mybir.MatmulPerfMode.DoubleRow

    xT = nc.dram_tensor("xT", [D, ROWS], bf16, kind="ExternalInput")
    x8T = nc.dram_tensor("x8T", [D, ROWS], fp8, kind="ExternalInput")
    wqT = nc.dram_tensor("wqT", [D, D], bf16, kind="ExternalInput")
    wkT = nc.dram_tensor("wkT", [D, D], bf16, kind="ExternalInput")
    wv8T = nc.dram_tensor("wv8T", [D, D], fp8, kind="ExternalInput")
    wo8T = nc.dram_tensor("wo8T", [D, D], fp8, kind="ExternalInput")
    cst = nc.dram_tensor("cst", [128, NCONST], f32, kind="ExternalInput")
    out = nc.dram_tensor("out", [D, TOK], f32, kind="ExternalOutput")

    with TileContext(nc) as tc:
        with tc.tile_pool(name="persist", bufs=1) as pers:
            vtile = [pers.tile([128, NUM_HEADS * VBLK], bf16, name=f"vt{r}")
                     for r in range(NVC)]
            for r in range(NVC):
                rows = min(128, ROWS - 128 * r)
                dst = vtile[r][0:rows, :].rearrange("p (b c) -> p b c", c=VBLK)
                nc.gpsimd.memset(dst[:, :, HEAD_DIM:VBLK], 1.0)

            # ---- input DMAs: per-chunk tiles, ordered by first use.
            # v runs first in fp8 DoubleRow: x/wv land as [128, 2, n] pairs.
            xp = [pers.tile([128, 2 * ROWS], fp8, name=f"xp{q}")
                  for q in range(KC // 2)]
            wvp = [pers.tile([128, 2 * D], fp8, name=f"wvp{q}")
                   for q in range(KC // 2)]
            xc = [pers.tile([128, ROWS], bf16, name=f"xc{c}")
                  for c in range(KC)]
            wkc = [pers.tile([128, D], bf16, name=f"wkc{c}")
                   for c in range(KC)]
            wqc = [pers.tile([128, D], bf16, name=f"wqc{c}")
                   for c in range(KC)]
            for q in range(KC // 2):
                nc.sync.dma_start(
                    xp[q][:].rearrange("p (k r) -> p k r", k=2),
                    x8T.ap()[256 * q:256 * (q + 1), :]
                    .rearrange("(k p) r -> p k r", p=128))
                nc.sync.dma_start(
                    wvp[q][:].rearrange("p (k n) -> p k n", k=2),
                    wv8T.ap()[256 * q:256 * (q + 1), :]
                    .rearrange("(k p) n -> p k n", p=128))
            cstt = pers.tile([128, NCONST], f32, name="cstt")
            nc.sync.dma_start(cstt[:], cst.ap())
            for c in range(KC):
                nc.sync.dma_start(xc[c][:], xT.ap()[c * 128:(c + 1) * 128, :])
                nc.sync.dma_start(wkc[c][:], wkT.ap()[c * 128:(c + 1) * 128, :])
            for c in range(KC):
                nc.sync.dma_start(wqc[c][:], wqT.ap()[c * 128:(c + 1) * 128, :])
            wop = [pers.tile([128, 2 * D], fp8, name=f"wop{t}")
                   for t in range(KC // 2)]
            for t in range(KC // 2):
                nc.sync.dma_start(
                    wop[t][:].rearrange("p (k n) -> p k n", k=2),
                    wo8T.ap()[256 * t:256 * (t + 1), :]
                    .rearrange("(k p) n -> p k n", p=128))

            bqt = cstt[0:HEAD_DIM, 0:NUM_HEADS]
            bo2t = cstt[:, 8:14]
            mkall = cstt[:, 14:NCONST].bitcast(bf16)   # [128, 384]
            masks = [mkall[:, 0:128], mkall[:, 128:320], mkall[0:64, 320:384]]

            qTh = [pers.tile([HEAD_DIM, TOK], bf16, name=f"qTh{h}")
                   for h in range(NUM_HEADS)]
            kTh = [pers.tile([HEAD_DIM, ROWS], bf16, name=f"kTh{h}")
                   for h in range(NUM_HEADS)]

            # ---- v projection (x-stationary, natural, fp8 DoubleRow)
            with tc.tile_pool(name="vpsum", bufs=2, space="PSUM") as vps:
                Copy = mybir.ActivationFunctionType.Copy
                for r in range(NVC):
                    rows = min(128, ROWS - 128 * r)
                    pv0 = vps.tile([128, 512], f32, tag="pv0", name="pv0")
                    pv1 = vps.tile([128, 256], f32, tag="pv1", name="pv1")
                    pv = [pv0, pv1]
                    for q in range(KC // 2):
                        for i, (nn, sz) in enumerate(((0, 512), (512, 256))):
                            nc.tensor.matmul(
                                pv[i][0:rows, 0:sz],
                                xp[q][:].rearrange("p (k r) -> p k r", k=2)
                                [:, :, 128 * r: 128 * r + rows],
                                wvp[q][:].rearrange("p (k n) -> p k n", k=2)
                                [:, :, nn: nn + sz],
                                start=(q == 0), stop=(q == KC // 2 - 1),
                                perf_mode=PM)
                    # wv is host-scaled by 64 (e4m3 normal range);
                    # descale on the PSUM->SBUF copies
                    dst = vtile[r][0:rows, :].rearrange(
                        "p (b c) -> p b c", c=VBLK)
                    nc.scalar.activation(
                        dst[:, 0:5, 0:HEAD_DIM],
                        pv0[0:rows, 0:5 * HEAD_DIM]
                        .rearrange("p (b c) -> p b c", c=HEAD_DIM),
                        Copy, scale=1.0 / 64)
                    # head 5 straddles the 512 boundary: 480:512 | 0:64
                    nc.scalar.activation(dst[:, 5, 0:32],
                                         pv0[0:rows, 480:512],
                                         Copy, scale=1.0 / 64)
                    nc.scalar.activation(dst[:, 5, 32:HEAD_DIM],
                                         pv1[0:rows, 0:64],
                                         Copy, scale=1.0 / 64)
                    nc.scalar.activation(
                        dst[:, 6:8, 0:HEAD_DIM],
                        pv1[0:rows, 64:64 + 2 * HEAD_DIM]
                        .rearrange("p (b c) -> p b c", c=HEAD_DIM),
                        Copy, scale=1.0 / 64)

            # ---- per-head k/q chains + attention, then out-projection
            with tc.tile_pool(name="apool", bufs=2) as apool, \
                 tc.tile_pool(name="upool", bufs=1) as upool, \
                 tc.tile_pool(name="opool", bufs=2) as opool, \
                 tc.tile_pool(name="kqpsum", bufs=2, space="PSUM") as kqpsum, \
                 tc.tile_pool(name="apsum", bufs=2, space="PSUM") as apsum, \
                 tc.tile_pool(name="spsum", bufs=2, space="PSUM") as spsum, \
                 tc.tile_pool(name="opsum", bufs=2, space="PSUM") as opsum:
                # packed fp8 ctx: per group, 3 pair tiles covering the 768
                # concatenated head dims as [128, 2, 512]
                ctxP = [[upool.tile([128, 2 * GRP], fp8, name=f"ctxP{g}_{t}",
                                    tag=f"ctxP{g}_{t}")
                         for t in range(KC // 2)] for g in range(NG)]

                def k_head(h):
                    for n0, sz in ((0, 512), (512, 512), (1024, 64)):
                        ps = kqpsum.tile([HEAD_DIM, 512], f32, tag="pqk",
                                         name="ps_k")
                        for c in range(KC):
                            nc.tensor.matmul(
                                ps[:, 0:sz],
                                wkc[c][:, h * HEAD_DIM:(h + 1) * HEAD_DIM],
                                xc[c][:, n0: n0 + sz],
                                start=(c == 0), stop=(c == KC - 1))
                        nc.scalar.copy(kTh[h][:, n0:n0 + sz], ps[:, 0:sz])

                def q_head(h):
                    for n0 in (0, 512):
                        ps = kqpsum.tile([HEAD_DIM, 512], f32, tag="pqk",
                                         name="ps_q")
                        for c in range(KC):
                            nc.tensor.matmul(
                                ps[:],
                                wqc[c][:, h * HEAD_DIM:(h + 1) * HEAD_DIM],
                                xc[c][:, OVERLAP + n0: OVERLAP + n0 + 512],
                                start=(c == 0), stop=(c == KC - 1))
                        nc.vector.tensor_scalar_add(
                            qTh[h][:, n0:n0 + 512], ps[:], bqt[:, h:h + 1])

                def attn_score(g, h, c, st):
                    """Score matmul (windowed) + exp + mask for one chunk."""
                    (k0, k1, w0, w1, mi) = CHUNKS[c]
                    ksz = k1 - k0
                    win = w1 - w0
                    s_ps = spsum.tile([128, 192], f32, tag="s", name="s_ps")
                    nc.tensor.matmul(
                        s_ps[0:ksz, 0:win],
                        kTh[h][:, GRP * g + k0: GRP * g + k1],
                        qTh[h][:, GRP * g + w0: GRP * g + w1],
                        start=True, stop=True)
                    ex = apool.tile([128, 192], bf16, tag="ex", name="ex",
                                    bufs=8)
                    nc.scalar.activation(
                        ex[0:ksz, 0:win], s_ps[0:ksz, 0:win],
                        mybir.ActivationFunctionType.Exp)
                    nc.vector.tensor_tensor(
                        out=ex[0:ksz, 0:win], in0=ex[0:ksz, 0:win],
                        in1=masks[mi][0:ksz, 0:win],
                        op=mybir.AluOpType.mult)
                    st[(g, c)] = ex

                def attn_ctx(g, h, c, st, ctx_ps):
                    (k0, k1, w0, w1, mi) = CHUNKS[c]
                    ksz = k1 - k0
                    win = w1 - w0
                    nc.tensor.matmul(
                        ctx_ps[:, w0:w1],
                        vtile[4 * g + c][0:ksz, h * VBLK:(h + 1) * VBLK],
                        st[(g, c)][0:ksz, 0:win],
                        start=(c == 0), stop=(c == len(CHUNKS) - 1),
                        skip_group_check=True)

                def normalize(g, h, ctx_ps):
                    # 1/d on DVE (no ACT table switch), broadcast on GPSIMD.
                    # The d row goes through SBUF: the custom DVE op does
                    # not read PSUM at a partition offset correctly. The
                    # 1/8 folds a x8 ctx pre-scale for the fp8 cast.
                    rl0 = apool.tile([1, GRP], f32, tag="rl0", name="rl0",
                                     bufs=2)
                    nc.scalar.activation(rl0[:], ctx_ps[HEAD_DIM:VBLK, :],
                                         mybir.ActivationFunctionType.Copy,
                                         scale=0.125)
                    rl = apool.tile([1, GRP], f32, tag="rl", name="rl", bufs=2)
                    nc.vector.reciprocal_approx_fast(rl[:], rl0[:])
                    rdb = apool.tile([HEAD_DIM, GRP], f32, tag="rdb",
                                     name="rdb", bufs=2)
                    nc.gpsimd.partition_broadcast(rdb[:], rl[:])
                    for (s0, s1, t, kk, d0) in CTX_PIECES[h]:
                        dst = ctxP[g][t][:].rearrange("p (k n) -> p k n", k=2)
                        nc.vector.tensor_tensor(
                            out=dst[d0:d0 + s1 - s0, kk, :],
                            in0=ctx_ps[s0:s1, :], in1=rdb[s0:s1, :],
                            op=mybir.AluOpType.mult)

                def outproj_mm(i, dc, t, op):
                    nc.tensor.matmul(
                        op[:],
                        wop[t][:].rearrange("p (k n) -> p k n", k=2)
                        [:, :, dc * 128:(dc + 1) * 128],
                        ctxP[i][t][:].rearrange("p (k n) -> p k n", k=2),
                        start=(t == 0), stop=(t == KC // 2 - 1),
                        perf_mode=PM)

                def outproj_fin(i, dc, op):
                    ost = opool.tile([128, 512], f32, tag="ost", name="ost")
                    # descale: ctx x8 and wo x64 fold to 1/512
                    nc.vector.tensor_scalar(ost[:], op[:], 1.0 / 512,
                                            bo2t[:, dc:dc + 1],
                                            op0=mybir.AluOpType.mult,
                                            op1=mybir.AluOpType.add)
                    nc.sync.dma_start(
                        out.ap()[dc * 128:(dc + 1) * 128,
                                 512 * i: 512 * (i + 1)], ost[:])

                def outproj_dc(i, dc):
                    op = opsum.tile([128, 512], f32, tag="po", name="ps_o")
                    for t in range(KC // 2):
                        outproj_mm(i, dc, t, op)
                    outproj_fin(i, dc, op)

                # Per-head software pipeline: head a's attention chunks are
                # woven between head a+1's five projection chains so the
                # exp/mask/normalize chains always have PE work to hide
                # under. Score of chunk c issues ~one chain before its ctx.
                for it in range(NUM_HEADS + 1):
                    a = it - 1
                    st = {}
                    ctxps = {}
                    if a >= 0:
                        ctxps[0] = apsum.tile([VBLK, GRP], f32, tag="ctx",
                                              name="ctx_ps")
                        ctxps[1] = apsum.tile([VBLK, GRP], f32, tag="ctx",
                                              name="ctx_ps")
                    units = ([[("s", 0, 0), ("s", 1, 0)]] +
                             [[("c", 0, c), ("c", 1, c),
                               ("s", 0, c + 1), ("s", 1, c + 1)]
                              for c in range(4)])

                    def emit_units(u):
                        for (kind, g, c) in u:
                            if kind == "s":
                                attn_score(g, a, c, st)
                            else:
                                attn_ctx(g, a, c, st, ctxps[g])

                    if it < NUM_HEADS:
                        chains = [("k", 0, 512), ("k", 512, 512),
                                  ("k", 1024, 64), ("q", 0, 512),
                                  ("q", 512, 512)]
                        for i, (kind, n0, sz) in enumerate(chains):
                            if kind == "k":
                                ps = kqpsum.tile([HEAD_DIM, 512], f32,
                                                 tag="pqk", name="ps_k")
                                for c in range(KC):
                                    nc.tensor.matmul(
                                        ps[:, 0:sz],
                                        wkc[c][:, it * HEAD_DIM:
                                               (it + 1) * HEAD_DIM],
                                        xc[c][:, n0: n0 + sz],
                                        start=(c == 0), stop=(c == KC - 1))
                                nc.scalar.copy(kTh[it][:, n0:n0 + sz],
                                               ps[:, 0:sz])
                            else:
                                ps = kqpsum.tile([HEAD_DIM, 512], f32,
                                                 tag="pqk", name="ps_q")
                                for c in range(KC):
                                    nc.tensor.matmul(
                                        ps[:],
                                        wqc[c][:, it * HEAD_DIM:
                                               (it + 1) * HEAD_DIM],
                                        xc[c][:, OVERLAP + n0:
                                              OVERLAP + n0 + 512],
                                        start=(c == 0), stop=(c == KC - 1))
                                nc.vector.tensor_scalar_add(
                                    qTh[it][:, n0:n0 + 512], ps[:],
                                    bqt[:, it:it + 1])
                            if a >= 0:
                                emit_units(units[i])
                        if a >= 0:
                            emit_units([("c", 0, 4), ("c", 1, 4)])
                            normalize(0, a, ctxps[0])
                            normalize(1, a, ctxps[1])
                    else:
                        # last head: weave the first out-projection pair's
                        # ready t-chunks into the attention chunk stream
                        opA = opsum.tile([128, 512], f32, tag="po",
                                         name="ps_o")
                        opB = opsum.tile([128, 512], f32, tag="po",
                                         name="ps_o")
                        emit_units(units[0])
                        outproj_mm(0, 0, 0, opA)
                        outproj_mm(0, 1, 0, opB)
                        emit_units(units[1])
                        outproj_mm(0, 0, 1, opA)
                        outproj_mm(0, 1, 1, opB)
                        for i in (2, 3):
                            emit_units(units[i])
                        emit_units(units[4])
                        emit_units([("c", 0, 4), ("c", 1, 4)])
                        normalize(0, a, ctxps[0])
                        normalize(1, a, ctxps[1])
                        outproj_mm(0, 0, 2, opA)
                        outproj_fin(0, 0, opA)
                        outproj_mm(0, 1, 2, opB)
                        outproj_fin(0, 1, opB)
                for dc in range(2, KC):
                    outproj_dc(0, dc)
                for dc in range(KC):
                    outproj_dc(1, dc)
    nc.compile()
    return nc


_program_cache = {}


def _get_program():
    if "nc" not in _program_cache:
        _program_cache["nc"] = _build_program()
    return _program_cache["nc"]


def _host_masks():
    # Three mask patterns: d = key - token offset within the chunk window.
    # m0 (first chunk): d = kk - mm; m1/m2 (later chunks): d = kk - mm + 64.
    import ml_dtypes
    masks = []
    for (nk, nw, off) in ((128, 128, 0), (128, 192, HALO), (64, 64, HALO)):
        kk, mm = np.meshgrid(np.arange(nk), np.arange(nw), indexing="ij")
        d = kk - mm + off
        valid = (d >= 0) & (d <= HALO) & (d % 4 == 0) & (d != OVERLAP)
        masks.append(valid.astype(ml_dtypes.bfloat16))
    return masks


def kernel(main, begin, end, in_proj_w, in_proj_b, out_proj_w, out_proj_b):
    import ml_dtypes
    from concourse.bass_utils import run_bass_kernel_spmd

    bf = ml_dtypes.bfloat16
    main = np.asarray(main, np.float32)
    begin = np.asarray(begin, np.float32)
    end = np.asarray(end, np.float32)
    in_proj_w = np.asarray(in_proj_w, np.float32)
    in_proj_b = np.asarray(in_proj_b, np.float32)
    out_proj_w = np.asarray(out_proj_w, np.float32)
    out_proj_b = np.asarray(out_proj_b, np.float32)

    D = EMBED_DIM
    scale = HEAD_DIM ** -0.5
    wq, wk, wv = in_proj_w[:D], in_proj_w[D:2 * D], in_proj_w[2 * D:]
    bq_, bv = in_proj_b[:D], in_proj_b[2 * D:3 * D]
    combined = np.concatenate([begin, main, end], axis=0)  # [N + 64, D]

    f8 = ml_dtypes.float8_e4m3
    wqT = np.ascontiguousarray(wq.T * scale).astype(bf)
    wkT = np.ascontiguousarray(wk.T).astype(bf)
    wv8T = np.ascontiguousarray(wv.T * 64.0).astype(f8)
    wo8T = np.ascontiguousarray(out_proj_w.T * 64.0).astype(f8)

    cst = np.zeros((128, NCONST), np.float32)
    cst[0:HEAD_DIM, 0:NUM_HEADS] = (bq_ * scale).reshape(NUM_HEADS, HEAD_DIM).T
    bo2 = out_proj_w @ bv + out_proj_b                      # [768]
    cst[:, 8:14] = bo2.reshape(KC, 128).T
    masks = _host_masks()
    mk = cst[:, 14:NCONST].view(bf)                         # [128, 384]
    mk[:, 0:128] = masks[0]
    mk[:, 128:320] = masks[1]
    mk[0:64, 320:384] = masks[2]

    shared = {"wqT": wqT, "wkT": wkT, "wv8T": wv8T, "wo8T": wo8T, "cst": cst}
    in_maps = []
    for c in range(N_CORES):
        xTc = np.ascontiguousarray(combined[c * TOK: c * TOK + ROWS].T)
        in_maps.append({**shared, "xT": xTc.astype(bf),
                        "x8T": xTc.astype(f8)})

    nc = _get_program()
    res = run_bass_kernel_spmd(nc, in_maps, core_ids=list(range(N_CORES)),
                               **_program_cache.get("run_kwargs", {}))
    _program_cache["last_result"] = res

    outp = np.empty((N_LINES, 2 * D), np.float32)
    outp[:, :D] = main
    for c in range(N_CORES):
        outp[c * TOK:(c + 1) * TOK, D:] = res.results[c]["out"].T
    return outp


# revision 31
# speedup vs baseline: 1.1794x; 1.1794x over previous
"""Trainium2 Bass kernel for dilated sliding-window attention (AttnWrapper).

Reference computation (all fp32):
  combined = [begin | main | end]                       # [8256, 768]
  keys[t]  = combined[t + 32 + off], off in +-{4..32}   # 16 dilated window keys
  q = (main @ wq.T + bq) * 96**-0.5
  k/v = keys @ w{k,v}.T + b{k,v}
  attn = softmax(q.k), ctx = attn.v, out = [main | ctx @ wo.T + bo]

Sharding: tokens across 8 cores (1024 each) with a 64-row halo of the
combined buffer; weights replicated. Each core computes attn_outT
[768, 1024]; the host transposes and concatenates with main.

Device-side notes (v2 — all-bf16 matmul pipeline):
 - All matmul operands bf16 (cast on host). bf16 runs 1 cycle/col at any
   size on PE (f32r needs >=256 cols), so score matmuls stream only each
   key-chunk's valid token window (128/192/192/192/64) instead of 512.
 - bk dropped (softmax shift invariance); bv folded into bo2 = wo@bv + bo;
   q scale and bq folded into wq/bq on the host.
 - A ones-column per V head block makes the ctx matmul also produce the
   softmax denominator (row 96 of the ctx PSUM tile). 1/d runs on DVE
   (reciprocal_approx_fast — keeps ACT on the Exp table, avoiding
   ~1.3us table reloads per Ln/Exp switch), broadcast on GPSIMD,
   multiply on DVE; all off the PE critical path.
 - Phases: v-proj, k-proj, then q-proj software-pipelined with group-0
   attention (head h's attention is emitted during head h+1's q chains),
   then group-1 attention interleaved with group-0 out-projection.
 - Per-chunk input DMAs ordered so the first v matmul starts ~8us in.
"""

import numpy as np

EMBED_DIM = 768
NUM_HEADS = 8
HEAD_DIM = 96
OVERLAP = 32
HALO = 2 * OVERLAP          # 64 extra combined rows per core
N_LINES = 8192
N_CORES = 8
TOK = N_LINES // N_CORES    # 1024 tokens per core
ROWS = TOK + HALO           # 1088 combined rows per core
GRP = 512                   # tokens per attention group
NG = TOK // GRP             # 2 groups
# key chunks (start, end), valid token windows (w0, w1), mask index
CHUNKS = [(0, 128, 0, 128, 0), (128, 256, 64, 256, 1), (256, 384, 192, 384, 1),
          (384, 512, 320, 512, 1), (512, 576, 448, 512, 2)]
VBLK = HEAD_DIM + 1         # 97: v head block + ones column
KC = EMBED_DIM // 128       # 6 contraction chunks of 128
NVC = (ROWS + 127) // 128   # 9 v row-chunks (8x128 + 64)
NCONST = 8 + 6 + 192        # bq | bo2 | masks(bf16 pairs)


def _ctx_pieces():
    """Per-head splits of the 96 ctx rows into segments legal on both the
    per-head source side (base = row) and the packed-768 destination side
    (base = global row % 128), given the partition-access rule: base must
    be 0/32/64/96 and the range must not cross the next alignment level
    (0->128, 32->32, 64->64, 96->32 max sizes)."""
    def legal(s, size):
        return size <= {0: 128, 32: 32, 64: 64, 96: 32}.get(s, -1)

    out = []
    for h in range(NUM_HEADS):
        g0 = HEAD_DIM * h
        merged = []
        for a in range(g0, g0 + HEAD_DIM, 32):
            if merged:
                m0, m1 = merged[-1]
                if (m0 // 128 == (a + 31) // 128
                        and legal(m0 % 128, a + 32 - m0)
                        and legal(m0 - g0, a + 32 - m0)):
                    merged[-1] = (m0, a + 32)
                    continue
            merged.append((a, a + 32))
        out.append([(a - g0, b - g0, a // 256, (a // 128) % 2, a % 128)
                    for a, b in merged])
    return out


CTX_PIECES = _ctx_pieces()


def _build_program():
    import concourse.bacc as bacc
    import concourse.mybir as mybir
    from concourse.tile import TileContext

    f32 = mybir.dt.float32
    bf16 = mybir.dt.bfloat16
    D = EMBED_DIM

    nc = bacc.Bacc("TRN2", target_bir_lowering=False, debug=False,
                   enable_asserts=False, num_devices=N_CORES)

    fp8 = mybir.dt.float8e4
    PM = mybir.MatmulPerfMode.DoubleRow

    xT = nc.dram_tensor("xT", [D, ROWS], bf16, kind="ExternalInput")
    x8T = nc.dram_tensor("x8T", [D, ROWS], fp8, kind="ExternalInput")
    wqT = nc.dram_tensor("wqT", [D, D], bf16, kind="ExternalInput")
    wkT = nc.dram_tensor("wkT", [D, D], bf16, kind="ExternalInput")
    wv8T = nc.dram_tensor("wv8T", [D, D], fp8, kind="ExternalInput")
    wo8T = nc.dram_tensor("wo8T", [D, D], fp8, kind="ExternalInput")
    cst = nc.dram_tensor("cst", [128, NCONST], f32, kind="ExternalInput")
    out = nc.dram_tensor("out", [D, TOK], f32, kind="ExternalOutput")

    with TileContext(nc) as tc:
        with tc.tile_pool(name="persist", bufs=1) as pers:
            vtile = [pers.tile([128, NUM_HEADS * VBLK], bf16, name=f"vt{r}")
                     for r in range(NVC)]
            for r in range(NVC):
                rows = min(128, ROWS - 128 * r)
                dst = vtile[r][0:rows, :].rearrange("p (b c) -> p b c", c=VBLK)
                nc.gpsimd.memset(dst[:, :, HEAD_DIM:VBLK], 1.0)

            # ---- input DMAs: per-chunk tiles, ordered by first use.
            # v runs first in fp8 DoubleRow: x/wv land as [128, 2, n] pairs.
            xp = [pers.tile([128, 2 * ROWS], fp8, name=f"xp{q}")
                  for q in range(KC // 2)]
            wvp = [pers.tile([128, 2 * D], fp8, name=f"wvp{q}")
                   for q in range(KC // 2)]
            xc = [pers.tile([128, ROWS], bf16, name=f"xc{c}")
                  for c in range(KC)]
            wkc = [pers.tile([128, D], bf16, name=f"wkc{c}")
                   for c in range(KC)]
            wqc = [pers.tile([128, D], bf16, name=f"wqc{c}")
                   for c in range(KC)]
            for q in range(KC // 2):
                nc.sync.dma_start(
                    xp[q][:].rearrange("p (k r) -> p k r", k=2),
                    x8T.ap()[256 * q:256 * (q + 1), :]
                    .rearrange("(k p) r -> p k r", p=128))
                nc.sync.dma_start(
                    wvp[q][:].rearrange("p (k n) -> p k n", k=2),
                    wv8T.ap()[256 * q:256 * (q + 1), :]
                    .rearrange("(k p) n -> p k n", p=128))
            cstt = pers.tile([128, NCONST], f32, name="cstt")
            nc.sync.dma_start(cstt[:], cst.ap())
            for c in range(KC):
                nc.sync.dma_start(xc[c][:], xT.ap()[c * 128:(c + 1) * 128, :])
                nc.sync.dma_start(wkc[c][:], wkT.ap()[c * 128:(c + 1) * 128, :])
            for c in range(KC):
                nc.sync.dma_start(wqc[c][:], wqT.ap()[c * 128:(c + 1) * 128, :])
            wop = [pers.tile([128, 2 * D], fp8, name=f"wop{t}")
                   for t in range(KC // 2)]
            for t in range(KC // 2):
                nc.sync.dma_start(
                    wop[t][:].rearrange("p (k n) -> p k n", k=2),
                    wo8T.ap()[256 * t:256 * (t + 1), :]
                    .rearrange("(k p) n -> p k n", p=128))

            bqt = cstt[0:HEAD_DIM, 0:NUM_HEADS]
            bo2t = cstt[:, 8:14]
            mkall = cstt[:, 14:NCONST].bitcast(bf16)   # [128, 384]
            masks = [mkall[:, 0:128], mkall[:, 128:320], mkall[0:64, 320:384]]

            qTh = [pers.tile([HEAD_DIM, TOK], bf16, name=f"qTh{h}")
                   for h in range(NUM_HEADS)]
            kTh = [pers.tile([HEAD_DIM, ROWS], bf16, name=f"kTh{h}")
                   for h in range(NUM_HEADS)]

            # ---- v projection (x-stationary, natural, fp8 DoubleRow)
            with tc.tile_pool(name="vpsum", bufs=2, space="PSUM") as vps:
                Copy = mybir.ActivationFunctionType.Copy
                for r in range(NVC):
                    rows = min(128, ROWS - 128 * r)
                    pv0 = vps.tile([128, 512], f32, tag="pv0", name="pv0")
                    pv1 = vps.tile([128, 256], f32, tag="pv1", name="pv1")
                    pv = [pv0, pv1]
                    for q in range(KC // 2):
                        for i, (nn, sz) in enumerate(((0, 512), (512, 256))):
                            nc.tensor.matmul(
                                pv[i][0:rows, 0:sz],
                                xp[q][:].rearrange("p (k r) -> p k r", k=2)
                                [:, :, 128 * r: 128 * r + rows],
                                wvp[q][:].rearrange("p (k n) -> p k n", k=2)
                                [:, :, nn: nn + sz],
                                start=(q == 0), stop=(q == KC // 2 - 1),
                                perf_mode=PM)
                    # wv is host-scaled by 64 (e4m3 normal range);
                    # descale on the PSUM->SBUF copies
                    dst = vtile[r][0:rows, :].rearrange(
                        "p (b c) -> p b c", c=VBLK)
                    nc.scalar.activation(
                        dst[:, 0:5, 0:HEAD_DIM],
                        pv0[0:rows, 0:5 * HEAD_DIM]
                        .rearrange("p (b c) -> p b c", c=HEAD_DIM),
                        Copy, scale=1.0 / 64)
                    # head 5 straddles the 512 boundary: 480:512 | 0:64
                    nc.scalar.activation(dst[:, 5, 0:32],
                                         pv0[0:rows, 480:512],
                                         Copy, scale=1.0 / 64)
                    nc.scalar.activation(dst[:, 5, 32:HEAD_DIM],
                                         pv1[0:rows, 0:64],
                                         Copy, scale=1.0 / 64)
                    nc.scalar.activation(
                        dst[:, 6:8, 0:HEAD_DIM],
                        pv1[0:rows, 64:64 + 2 * HEAD_DIM]
                        .rearrange("p (b c) -> p b c", c=HEAD_DIM),
                        Copy, scale=1.0 / 64)

            # ---- per-head k/q chains + attention, then out-projection
            with tc.tile_pool(name="apool", bufs=2) as apool, \
                 tc.tile_pool(name="upool", bufs=1) as upool, \
                 tc.tile_pool(name="opool", bufs=2) as opool, \
                 tc.tile_pool(name="kqpsum", bufs=2, space="PSUM") as kqpsum, \
                 tc.tile_pool(name="apsum", bufs=2, space="PSUM") as apsum, \
                 tc.tile_pool(name="spsum", bufs=2, space="PSUM") as spsum, \
                 tc.tile_pool(name="opsum", bufs=2, space="PSUM") as opsum:
                # packed fp8 ctx: per group, 3 pair tiles covering the 768
                # concatenated head dims as [128, 2, 512]
                ctxP = [[upool.tile([128, 2 * GRP], fp8, name=f"ctxP{g}_{t}",
                                    tag=f"ctxP{g}_{t}")
                         for t in range(KC // 2)] for g in range(NG)]

                def k_head(h):
                    for n0, sz in ((0, 512), (512, 512), (1024, 64)):
                        ps = kqpsum.tile([HEAD_DIM, 512], f32, tag="pqk",
                                         name="ps_k")
                        for c in range(KC):
                            nc.tensor.matmul(
                                ps[:, 0:sz],
                                wkc[c][:, h * HEAD_DIM:(h + 1) * HEAD_DIM],
                                xc[c][:, n0: n0 + sz],
                                start=(c == 0), stop=(c == KC - 1))
                        nc.scalar.copy(kTh[h][:, n0:n0 + sz], ps[:, 0:sz])

                def q_head(h):
                    for n0 in (0, 512):
                        ps = kqpsum.tile([HEAD_DIM, 512], f32, tag="pqk",
                                         name="ps_q")
                        for c in range(KC):
                            nc.tensor.matmul(
                                ps[:],
                                wqc[c][:, h * HEAD_DIM:(h + 1) * HEAD_DIM],
                                xc[c][:, OVERLAP + n0: OVERLAP + n0 + 512],
                                start=(c == 0), stop=(c == KC - 1))
                        nc.vector.tensor_scalar_add(
                            qTh[h][:, n0:n0 + 512], ps[:], bqt[:, h:h + 1])

                def attn_score(g, h, c, st):
                    """Score matmul (windowed) + exp + mask for one chunk."""
                    (k0, k1, w0, w1, mi) = CHUNKS[c]
                    ksz = k1 - k0
                    win = w1 - w0
                    s_ps = spsum.tile([128, 192], f32, tag="s", name="s_ps")
                    nc.tensor.matmul(
                        s_ps[0:ksz, 0:win],
                        kTh[h][:, GRP * g + k0: GRP * g + k1],
                        qTh[h][:, GRP * g + w0: GRP * g + w1],
                        start=True, stop=True)
                    ex = apool.tile([128, 192], bf16, tag="ex", name="ex",
                                    bufs=8)
                    nc.scalar.activation(
                        ex[0:ksz, 0:win], s_ps[0:ksz, 0:win],
                        mybir.ActivationFunctionType.Exp)
                    nc.vector.tensor_tensor(
                        out=ex[0:ksz, 0:win], in0=ex[0:ksz, 0:win],
                        in1=masks[mi][0:ksz, 0:win],
                        op=mybir.AluOpType.mult)
                    st[(g, c)] = ex

                def attn_ctx(g, h, c, st, ctx_ps):
                    (k0, k1, w0, w1, mi) = CHUNKS[c]
                    ksz = k1 - k0
                    win = w1 - w0
                    nc.tensor.matmul(
                        ctx_ps[:, w0:w1],
                        vtile[4 * g + c][0:ksz, h * VBLK:(h + 1) * VBLK],
                        st[(g, c)][0:ksz, 0:win],
                        start=(c == 0), stop=(c == len(CHUNKS) - 1),
                        skip_group_check=True)

                def normalize(g, h, ctx_ps):
                    # 1/d on DVE (no ACT table switch), broadcast on GPSIMD.
                    # The d row goes through SBUF: the custom DVE op does
                    # not read PSUM at a partition offset correctly. The
                    # 1/8 folds a x8 ctx pre-scale for the fp8 cast.
                    rl0 = apool.tile([1, GRP], f32, tag="rl0", name="rl0",
                                     bufs=2)
                    nc.scalar.activation(rl0[:], ctx_ps[HEAD_DIM:VBLK, :],
                                         mybir.ActivationFunctionType.Copy,
                                         scale=0.125)
                    rl = apool.tile([1, GRP], f32, tag="rl", name="rl", bufs=2)
                    nc.vector.reciprocal_approx_fast(rl[:], rl0[:])
                    rdb = apool.tile([HEAD_DIM, GRP], f32, tag="rdb",
                                     name="rdb", bufs=2)
                    nc.gpsimd.partition_broadcast(rdb[:], rl[:])
                    for (s0, s1, t, kk, d0) in CTX_PIECES[h]:
                        dst = ctxP[g][t][:].rearrange("p (k n) -> p k n", k=2)
                        nc.vector.tensor_tensor(
                            out=dst[d0:d0 + s1 - s0, kk, :],
                            in0=ctx_ps[s0:s1, :], in1=rdb[s0:s1, :],
                            op=mybir.AluOpType.mult)

                def outproj_mm(i, dc, t, op):
                    nc.tensor.matmul(
                        op[:],
                        wop[t][:].rearrange("p (k n) -> p k n", k=2)
                        [:, :, dc * 128:(dc + 1) * 128],
                        ctxP[i][t][:].rearrange("p (k n) -> p k n", k=2),
                        start=(t == 0), stop=(t == KC // 2 - 1),
                        perf_mode=PM)

                def outproj_fin(i, dc, op):
                    ost = opool.tile([128, 512], f32, tag="ost", name="ost")
                    # descale: ctx x8 and wo x64 fold to 1/512
                    nc.vector.tensor_scalar(ost[:], op[:], 1.0 / 512,
                                            bo2t[:, dc:dc + 1],
                                            op0=mybir.AluOpType.mult,
                                            op1=mybir.AluOpType.add)
                    nc.sync.dma_start(
                        out.ap()[dc * 128:(dc + 1) * 128,
                                 512 * i: 512 * (i + 1)], ost[:])

                def outproj_dc(i, dc):
                    op = opsum.tile([128, 512], f32, tag="po", name="ps_o")
                    for t in range(KC // 2):
                        outproj_mm(i, dc, t, op)
                    outproj_fin(i, dc, op)

                def attention_both(a, ops=None):
                    """Both groups' attention for head a, chunk-interleaved
                    so every ctx matmul trails its score by ~2 PE slots
                    (exp+mask latency). `ops` optionally weaves ready
                    out-projection matmuls into the stream (last head)."""
                    st = {}
                    ctxps = {
                        0: apsum.tile([VBLK, GRP], f32, tag="ctx",
                                      name="ctx_ps"),
                        1: apsum.tile([VBLK, GRP], f32, tag="ctx",
                                      name="ctx_ps"),
                    }
                    stream = [("s", 0, 0), ("s", 0, 1), ("c", 0, 0),
                              ("s", 0, 2), ("c", 0, 1), ("s", 0, 3),
                              ("c", 0, 2), ("s", 0, 4), ("c", 0, 3),
                              ("s", 1, 0), ("c", 0, 4), ("n", 0, 0),
                              ("s", 1, 1), ("c", 1, 0), ("s", 1, 2),
                              ("c", 1, 1), ("s", 1, 3), ("c", 1, 2),
                              ("s", 1, 4), ("c", 1, 3), ("c", 1, 4),
                              ("n", 1, 0)]
                    for idx, (kind, g, c) in enumerate(stream):
                        if kind == "s":
                            attn_score(g, a, c, st)
                        elif kind == "c":
                            attn_ctx(g, a, c, st, ctxps[g])
                        else:
                            normalize(g, a, ctxps[g])
                        if ops and idx in ops:
                            for fn in ops[idx]:
                                fn()
                    return ctxps

                # head a's attention runs while head a+1's chains occupy
                # the PE; chains form unbroken MM runs (pstate-friendly)
                for it in range(NUM_HEADS):
                    k_head(it)
                    q_head(it)
                    if it > 0:
                        attention_both(it - 1)
                # last head: weave the first out-projection pair's ready
                # t-chunks into its attention stream
                opA = opsum.tile([128, 512], f32, tag="po", name="ps_o")
                opB = opsum.tile([128, 512], f32, tag="po", name="ps_o")
                ops = {
                    4: [lambda: outproj_mm(0, 0, 0, opA),
                        lambda: outproj_mm(0, 1, 0, opB)],
                    15: [lambda: outproj_mm(0, 0, 1, opA),
                         lambda: outproj_mm(0, 1, 1, opB)],
                }
                attention_both(NUM_HEADS - 1, ops)
                outproj_mm(0, 0, 2, opA)
                outproj_fin(0, 0, opA)
                outproj_mm(0, 1, 2, opB)
                outproj_fin(0, 1, opB)
                for dc in range(2, KC):
                    outproj_dc(0, dc)
                for dc in range(KC):
                    outproj_dc(1, dc)
    nc.compile()
    return nc


_program_cache = {}


def _get_program():
    if "nc" not in _program_cache:
        _program_cache["nc"] = _build_program()
    return _program_cache["nc"]


def _host_masks():
    # Three mask patterns: d = key - token offset within the chunk window.
    # m0 (first chunk): d = kk - mm; m1/m2 (later chunks): d = kk - mm + 64.
    import ml_dtypes
    masks = []
    for (nk, nw, off) in ((128, 128, 0), (128, 192, HALO), (64, 64, HALO)):
        kk, mm = np.meshgrid(np.arange(nk), np.arange(nw), indexing="ij")
        d = kk - mm + off
        valid = (d >= 0) & (d <= HALO) & (d % 4 == 0) & (d != OVERLAP)
        masks.append(valid.astype(ml_dtypes.bfloat16))
    return masks


def kernel(main, begin, end, in_proj_w, in_proj_b, out_proj_w, out_proj_b):
    import ml_dtypes
    from concourse.bass_utils import run_bass_kernel_spmd

    bf = ml_dtypes.bfloat16
    main = np.asarray(main, np.float32)
    begin = np.asarray(begin, np.float32)
    end = np.asarray(end, np.float32)
    in_proj_w = np.asarray(in_proj_w, np.float32)
    in_proj_b = np.asarray(in_proj_b, np.float32)
    out_proj_w = np.asarray(out_proj_w, np.float32)
    out_proj_b = np.asarray(out_proj_b, np.float32)

    D = EMBED_DIM
    scale = HEAD_DIM ** -0.5
    wq, wk, wv = in_proj_w[:D], in_proj_w[D:2 * D], in_proj_w[2 * D:]
    bq_, bv = in_proj_b[:D], in_proj_b[2 * D:3 * D]
    combined = np.concatenate([begin, main, end], axis=0)  # [N + 64, D]

    f8 = ml_dtypes.float8_e4m3
    wqT = np.ascontiguousarray(wq.T * scale).astype(bf)
    wkT = np.ascontiguousarray(wk.T).astype(bf)
    wv8T = np.ascontiguousarray(wv.T * 64.0).astype(f8)
    wo8T = np.ascontiguousarray(out_proj_w.T * 64.0).astype(f8)

    cst = np.zeros((128, NCONST), np.float32)
    cst[0:HEAD_DIM, 0:NUM_HEADS] = (bq_ * scale).reshape(NUM_HEADS, HEAD_DIM).T
    bo2 = out_proj_w @ bv + out_proj_b                      # [768]
    cst[:, 8:14] = bo2.reshape(KC, 128).T
    masks = _host_masks()
    mk = cst[:, 14:NCONST].view(bf)                         # [128, 384]
    mk[:, 0:128] = masks[0]
    mk[:, 128:320] = masks[1]
    mk[0:64, 320:384] = masks[2]

    shared = {"wqT": wqT, "wkT": wkT, "wv8T": wv8T, "wo8T": wo8T, "cst": cst}
    in_maps = []
    for c in range(N_CORES):
        xTc = np.ascontiguousarray(combined[c * TOK: c * TOK + ROWS].T)
        in_maps.append({**shared, "xT": xTc.astype(bf),
                        "x8T": xTc.astype(f8)})

    nc = _get_program()
    res = run_bass_kernel_spmd(nc, in_maps, core_ids=list(range(N_CORES)),
                               **_program_cache.get("run_kwargs", {}))
    _program_cache["last_result"] = res

    outp = np.empty((N_LINES, 2 * D), np.float32)
    outp[:, :D] = main
    for c in range(N_CORES):
        outp[c * TOK:(c + 1) * TOK, D:] = res.results[c]["out"].T
    return outp


# revision 42
# speedup vs baseline: 1.2549x; 1.0640x over previous
"""Trainium2 Bass kernel for dilated sliding-window attention (AttnWrapper).

Reference computation (all fp32):
  combined = [begin | main | end]                       # [8256, 768]
  keys[t]  = combined[t + 32 + off], off in +-{4..32}   # 16 dilated window keys
  q = (main @ wq.T + bq) * 96**-0.5
  k/v = keys @ w{k,v}.T + b{k,v}
  attn = softmax(q.k), ctx = attn.v, out = [main | ctx @ wo.T + bo]

Sharding: tokens across 8 cores (1024 each) with a 64-row halo of the
combined buffer; weights replicated. Each core computes attn_outT
[768, 1024]; the host transposes and concatenates with main.

Device-side notes (v2 — all-bf16 matmul pipeline):
 - All matmul operands bf16 (cast on host). bf16 runs 1 cycle/col at any
   size on PE (f32r needs >=256 cols), so score matmuls stream only each
   key-chunk's valid token window (128/192/192/192/64) instead of 512.
 - bk dropped (softmax shift invariance); bv folded into bo2 = wo@bv + bo;
   q scale and bq folded into wq/bq on the host.
 - A ones-column per V head block makes the ctx matmul also produce the
   softmax denominator (row 96 of the ctx PSUM tile). 1/d runs on DVE
   (reciprocal_approx_fast — keeps ACT on the Exp table, avoiding
   ~1.3us table reloads per Ln/Exp switch), broadcast on GPSIMD,
   multiply on DVE; all off the PE critical path.
 - Phases: v-proj, k-proj, then q-proj software-pipelined with group-0
   attention (head h's attention is emitted during head h+1's q chains),
   then group-1 attention interleaved with group-0 out-projection.
 - Per-chunk input DMAs ordered so the first v matmul starts ~8us in.
"""

import numpy as np

EMBED_DIM = 768
NUM_HEADS = 8
HEAD_DIM = 96
OVERLAP = 32
HALO = 2 * OVERLAP          # 64 extra combined rows per core
N_LINES = 8192
N_CORES = 8
TOK = N_LINES // N_CORES    # 1024 tokens per core
ROWS = TOK + HALO           # 1088 combined rows per core
GRP = 512                   # tokens per attention group
NG = TOK // GRP             # 2 groups
# key chunks (start, end), valid token windows (w0, w1), mask index
CHUNKS = [(0, 128, 0, 128, 0), (128, 256, 64, 256, 1), (256, 384, 192, 384, 1),
          (384, 512, 320, 512, 1), (512, 576, 448, 512, 2)]
VBLK = HEAD_DIM + 1         # 97: v head block + ones column
KC = EMBED_DIM // 128       # 6 contraction chunks of 128
NVC = (ROWS + 127) // 128   # 9 v row-chunks (8x128 + 64)
NCONST = 8 + 6 + 192        # bq | bo2 | masks(bf16 pairs)


def _ctx_pieces():
    """Per-head splits of the 96 ctx rows into segments legal on both the
    per-head source side (base = row) and the packed-768 destination side
    (base = global row % 128), given the partition-access rule: base must
    be 0/32/64/96 and the range must not cross the next alignment level
    (0->128, 32->32, 64->64, 96->32 max sizes)."""
    def legal(s, size):
        return size <= {0: 128, 32: 32, 64: 64, 96: 32}.get(s, -1)

    out = []
    for h in range(NUM_HEADS):
        g0 = HEAD_DIM * h
        merged = []
        for a in range(g0, g0 + HEAD_DIM, 32):
            if merged:
                m0, m1 = merged[-1]
                if (m0 // 128 == (a + 31) // 128
                        and legal(m0 % 128, a + 32 - m0)
                        and legal(m0 - g0, a + 32 - m0)):
                    merged[-1] = (m0, a + 32)
                    continue
            merged.append((a, a + 32))
        out.append([(a - g0, b - g0, a // 256, (a // 128) % 2, a % 128)
                    for a, b in merged])
    return out


CTX_PIECES = _ctx_pieces()


def _build_program():
    import concourse.bacc as bacc
    import concourse.mybir as mybir
    from concourse.tile import TileContext

    f32 = mybir.dt.float32
    bf16 = mybir.dt.bfloat16
    D = EMBED_DIM

    nc = bacc.Bacc("TRN2", target_bir_lowering=False, debug=False,
                   enable_asserts=False, num_devices=N_CORES)

    fp8 = mybir.dt.float8e4
    PM = mybir.MatmulPerfMode.DoubleRow

    xT = nc.dram_tensor("xT", [D, ROWS], bf16, kind="ExternalInput")
    # x8 and wv8 side by side: one DMA per 256-row pair feeds the v matmul
    xv8T = nc.dram_tensor("xv8T", [D, ROWS + D], fp8, kind="ExternalInput")
    wqT = nc.dram_tensor("wqT", [D, D], bf16, kind="ExternalInput")
    wkT = nc.dram_tensor("wkT", [D, D], bf16, kind="ExternalInput")
    wo8T = nc.dram_tensor("wo8T", [D, D], fp8, kind="ExternalInput")
    cst = nc.dram_tensor("cst", [128, NCONST], f32, kind="ExternalInput")
    out = nc.dram_tensor("out", [D, TOK], f32, kind="ExternalOutput")

    with TileContext(nc) as tc:
        with tc.tile_pool(name="persist", bufs=1) as pers:
            vtile = [pers.tile([128, NUM_HEADS * VBLK], bf16, name=f"vt{r}")
                     for r in range(NVC)]
            for r in range(NVC):
                rows = min(128, ROWS - 128 * r)
                dst = vtile[r][0:rows, :].rearrange("p (b c) -> p b c", c=VBLK)
                nc.gpsimd.memset(dst[:, :, HEAD_DIM:VBLK], 1.0)

            # ---- input DMAs: per-chunk tiles, ordered by first use.
            # v runs first in fp8 DoubleRow: x|wv land as [128, 2, n] pairs.
            RD = ROWS + D
            xvp = [pers.tile([128, 2 * RD], fp8, name=f"xvp{q}")
                   for q in range(KC // 2)]
            xc = [pers.tile([128, ROWS], bf16, name=f"xc{c}")
                  for c in range(KC)]
            wkc = [pers.tile([128, D], bf16, name=f"wkc{c}")
                   for c in range(KC)]
            wqc = [pers.tile([128, D], bf16, name=f"wqc{c}")
                   for c in range(KC)]
            for q in range(KC // 2):
                nc.sync.dma_start(
                    xvp[q][:].rearrange("p (k r) -> p k r", k=2),
                    xv8T.ap()[256 * q:256 * (q + 1), :]
                    .rearrange("(k p) r -> p k r", p=128))
            for c in range(KC):
                nc.sync.dma_start(xc[c][:], xT.ap()[c * 128:(c + 1) * 128, :])
                nc.sync.dma_start(wkc[c][:], wkT.ap()[c * 128:(c + 1) * 128, :])
            cstt = pers.tile([128, NCONST], f32, name="cstt")
            nc.sync.dma_start(cstt[:], cst.ap())
            for c in range(KC):
                nc.sync.dma_start(wqc[c][:], wqT.ap()[c * 128:(c + 1) * 128, :])
            wop = [pers.tile([128, 2 * D], fp8, name=f"wop{t}")
                   for t in range(KC // 2)]
            for t in range(KC // 2):
                nc.sync.dma_start(
                    wop[t][:].rearrange("p (k n) -> p k n", k=2),
                    wo8T.ap()[256 * t:256 * (t + 1), :]
                    .rearrange("(k p) n -> p k n", p=128))

            bqt = cstt[0:HEAD_DIM, 0:NUM_HEADS]
            bo2t = cstt[:, 8:14]
            mkall = cstt[:, 14:NCONST].bitcast(bf16)   # [128, 384]
            masks = [mkall[:, 0:128], mkall[:, 128:320], mkall[0:64, 320:384]]

            qTh = [pers.tile([HEAD_DIM, TOK], bf16, name=f"qTh{h}")
                   for h in range(NUM_HEADS)]
            kTh = [pers.tile([HEAD_DIM, ROWS], bf16, name=f"kTh{h}")
                   for h in range(NUM_HEADS)]

            # ---- v projection (x-stationary, natural, fp8 DoubleRow)
            with tc.tile_pool(name="vpsum", bufs=2, space="PSUM") as vps:
                Copy = mybir.ActivationFunctionType.Copy
                for r in range(NVC):
                    rows = min(128, ROWS - 128 * r)
                    pv0 = vps.tile([128, 512], f32, tag="pv0", name="pv0")
                    pv1 = vps.tile([128, 256], f32, tag="pv1", name="pv1")
                    pv = [pv0, pv1]
                    for q in range(KC // 2):
                        xv = xvp[q][:].rearrange("p (k r) -> p k r", k=2)
                        for i, (nn, sz) in enumerate(((0, 512), (512, 256))):
                            nc.tensor.matmul(
                                pv[i][0:rows, 0:sz],
                                xv[:, :, 128 * r: 128 * r + rows],
                                xv[:, :, ROWS + nn: ROWS + nn + sz],
                                start=(q == 0), stop=(q == KC // 2 - 1),
                                perf_mode=PM)
                    # wv is host-scaled by 64 (e4m3 normal range);
                    # descale on the PSUM->SBUF copies
                    dst = vtile[r][0:rows, :].rearrange(
                        "p (b c) -> p b c", c=VBLK)
                    nc.scalar.activation(
                        dst[:, 0:5, 0:HEAD_DIM],
                        pv0[0:rows, 0:5 * HEAD_DIM]
                        .rearrange("p (b c) -> p b c", c=HEAD_DIM),
                        Copy, scale=1.0 / 64)
                    # head 5 straddles the 512 boundary: 480:512 | 0:64
                    nc.scalar.activation(dst[:, 5, 0:32],
                                         pv0[0:rows, 480:512],
                                         Copy, scale=1.0 / 64)
                    nc.scalar.activation(dst[:, 5, 32:HEAD_DIM],
                                         pv1[0:rows, 0:64],
                                         Copy, scale=1.0 / 64)
                    nc.scalar.activation(
                        dst[:, 6:8, 0:HEAD_DIM],
                        pv1[0:rows, 64:64 + 2 * HEAD_DIM]
                        .rearrange("p (b c) -> p b c", c=HEAD_DIM),
                        Copy, scale=1.0 / 64)

            # ---- per-head k/q chains + attention, then out-projection
            with tc.tile_pool(name="apool", bufs=2) as apool, \
                 tc.tile_pool(name="upool", bufs=1) as upool, \
                 tc.tile_pool(name="opool", bufs=4) as opool:
                # packed fp8 ctx: per group, 3 pair tiles covering the 768
                # concatenated head dims as [128, 2, 512]
                ctxP = [[upool.tile([128, 2 * GRP], fp8, name=f"ctxP{g}_{t}",
                                    tag=f"ctxP{g}_{t}")
                         for t in range(KC // 2)] for g in range(NG)]

                def k_head(h):
                    for n0, sz in ((0, 512), (512, 512), (1024, 64)):
                        ps = kqpsum.tile([HEAD_DIM, 512], f32, tag="pqk",
                                         name="ps_k")
                        for c in range(KC):
                            nc.tensor.matmul(
                                ps[:, 0:sz],
                                wkc[c][:, h * HEAD_DIM:(h + 1) * HEAD_DIM],
                                xc[c][:, n0: n0 + sz],
                                start=(c == 0), stop=(c == KC - 1))
                        nc.scalar.copy(kTh[h][:, n0:n0 + sz], ps[:, 0:sz])

                def q_head(h):
                    for n0 in (0, 512):
                        ps = kqpsum.tile([HEAD_DIM, 512], f32, tag="pqk",
                                         name="ps_q")
                        for c in range(KC):
                            nc.tensor.matmul(
                                ps[:],
                                wqc[c][:, h * HEAD_DIM:(h + 1) * HEAD_DIM],
                                xc[c][:, OVERLAP + n0: OVERLAP + n0 + 512],
                                start=(c == 0), stop=(c == KC - 1))
                        nc.vector.tensor_scalar_add(
                            qTh[h][:, n0:n0 + 512], ps[:], bqt[:, h:h + 1])

                def attn_score(g, h, c, st):
                    """Score matmul (windowed) + exp + mask for one chunk."""
                    (k0, k1, w0, w1, mi) = CHUNKS[c]
                    ksz = k1 - k0
                    win = w1 - w0
                    s_ps = spsum.tile([128, 192], f32, tag="s", name="s_ps")
                    nc.tensor.matmul(
                        s_ps[0:ksz, 0:win],
                        kTh[h][:, GRP * g + k0: GRP * g + k1],
                        qTh[h][:, GRP * g + w0: GRP * g + w1],
                        start=True, stop=True)
                    ex = apool.tile([128, 192], bf16, tag="ex", name="ex",
                                    bufs=8)
                    nc.scalar.activation(
                        ex[0:ksz, 0:win], s_ps[0:ksz, 0:win],
                        mybir.ActivationFunctionType.Exp)
                    nc.vector.tensor_tensor(
                        out=ex[0:ksz, 0:win], in0=ex[0:ksz, 0:win],
                        in1=masks[mi][0:ksz, 0:win],
                        op=mybir.AluOpType.mult)
                    st[(g, c)] = ex

                def attn_ctx(g, h, c, st, ctx_ps):
                    (k0, k1, w0, w1, mi) = CHUNKS[c]
                    ksz = k1 - k0
                    win = w1 - w0
                    nc.tensor.matmul(
                        ctx_ps[:, w0:w1],
                        vtile[4 * g + c][0:ksz, h * VBLK:(h + 1) * VBLK],
                        st[(g, c)][0:ksz, 0:win],
                        start=(c == 0), stop=(c == len(CHUNKS) - 1),
                        skip_group_check=True)

                def normalize(g, h, ctx_ps):
                    # 1/d on DVE (no ACT table switch), broadcast on GPSIMD.
                    # The d row goes through SBUF: the custom DVE op does
                    # not read PSUM at a partition offset correctly. The
                    # 1/8 folds a x8 ctx pre-scale for the fp8 cast.
                    rl0 = apool.tile([1, GRP], f32, tag="rl0", name="rl0",
                                     bufs=2)
                    nc.scalar.activation(rl0[:], ctx_ps[HEAD_DIM:VBLK, :],
                                         mybir.ActivationFunctionType.Copy,
                                         scale=0.125)
                    rl = apool.tile([1, GRP], f32, tag="rl", name="rl", bufs=2)
                    nc.vector.reciprocal_approx_fast(rl[:], rl0[:])
                    rdb = apool.tile([HEAD_DIM, GRP], f32, tag="rdb",
                                     name="rdb", bufs=2)
                    nc.gpsimd.partition_broadcast(rdb[:], rl[:])
                    for (s0, s1, t, kk, d0) in CTX_PIECES[h]:
                        dst = ctxP[g][t][:].rearrange("p (k n) -> p k n", k=2)
                        nc.vector.tensor_tensor(
                            out=dst[d0:d0 + s1 - s0, kk, :],
                            in0=ctx_ps[s0:s1, :], in1=rdb[s0:s1, :],
                            op=mybir.AluOpType.mult)

                def outproj_mm(i, dc, t, op):
                    nc.tensor.matmul(
                        op[:],
                        wop[t][:].rearrange("p (k n) -> p k n", k=2)
                        [:, :, dc * 128:(dc + 1) * 128],
                        ctxP[i][t][:].rearrange("p (k n) -> p k n", k=2),
                        start=(t == 0), stop=(t == KC // 2 - 1),
                        perf_mode=PM)

                def outproj_fin(i, dc, op, j):
                    # plain PSUM->SBUF copy (DMA cannot read PSUM),
                    # alternating ACT/DVE; the host folds the 1/512
                    # descale + bo2 bias into its transpose pass
                    ost = opool.tile([128, 512], f32, tag="ost", name="ost")
                    if j % 2 == 0:
                        nc.scalar.copy(ost[:], op[:])
                    else:
                        nc.vector.tensor_copy(ost[:], op[:])
                    nc.sync.dma_start(
                        out.ap()[dc * 128:(dc + 1) * 128,
                                 512 * i: 512 * (i + 1)], ost[:])

                def attention_both(a, ops=None):
                    """Both groups' attention for head a, chunk-interleaved
                    so every ctx matmul trails its score by ~2 PE slots
                    (exp+mask latency). `ops` optionally weaves ready
                    out-projection matmuls into the stream (last head)."""
                    st = {}
                    ctxps = {
                        0: apsum.tile([VBLK, GRP], f32, tag="ctx",
                                      name="ctx_ps"),
                        1: apsum.tile([VBLK, GRP], f32, tag="ctx",
                                      name="ctx_ps"),
                    }
                    stream = [("s", 0, 0), ("s", 0, 1), ("c", 0, 0),
                              ("s", 0, 2), ("c", 0, 1), ("s", 0, 3),
                              ("c", 0, 2), ("s", 0, 4), ("c", 0, 3),
                              ("s", 1, 0), ("c", 0, 4), ("n", 0, 0),
                              ("s", 1, 1), ("c", 1, 0), ("s", 1, 2),
                              ("c", 1, 1), ("s", 1, 3), ("c", 1, 2),
                              ("s", 1, 4), ("c", 1, 3), ("c", 1, 4),
                              ("n", 1, 0)]
                    for idx, (kind, g, c) in enumerate(stream):
                        if kind == "s":
                            attn_score(g, a, c, st)
                        elif kind == "c":
                            attn_ctx(g, a, c, st, ctxps[g])
                        else:
                            normalize(g, a, ctxps[g])
                        if ops and idx in ops:
                            for fn in ops[idx]:
                                fn()
                    return ctxps

                # head a's attention runs while head a+1's chains occupy
                # the PE; chains form unbroken MM runs (pstate-friendly)
                with tc.tile_pool(name="kqpsum", bufs=2,
                                  space="PSUM") as kqpsum, \
                     tc.tile_pool(name="apsum", bufs=2,
                                  space="PSUM") as apsum, \
                     tc.tile_pool(name="spsum", bufs=2,
                                  space="PSUM") as spsum:
                    for it in range(NUM_HEADS):
                        k_head(it)
                        q_head(it)
                        if it > 0:
                            attention_both(it - 1)
                    attention_both(NUM_HEADS - 1)
                # attention PSUM freed: deep out-projection buffering
                with tc.tile_pool(name="opsum", bufs=6,
                                  space="PSUM") as opsum:
                    for j, (i, dc) in enumerate(
                            [(i, dc) for i in range(NG) for dc in range(KC)]):
                        op = opsum.tile([128, 512], f32, tag="po",
                                        name="ps_o")
                        for t in range(KC // 2):
                            outproj_mm(i, dc, t, op)
                        outproj_fin(i, dc, op, j)
    nc.compile()
    return nc


_program_cache = {}


def _get_program():
    if "nc" not in _program_cache:
        _program_cache["nc"] = _build_program()
    return _program_cache["nc"]


def _host_masks():
    # Three mask patterns: d = key - token offset within the chunk window.
    # m0 (first chunk): d = kk - mm; m1/m2 (later chunks): d = kk - mm + 64.
    import ml_dtypes
    masks = []
    for (nk, nw, off) in ((128, 128, 0), (128, 192, HALO), (64, 64, HALO)):
        kk, mm = np.meshgrid(np.arange(nk), np.arange(nw), indexing="ij")
        d = kk - mm + off
        valid = (d >= 0) & (d <= HALO) & (d % 4 == 0) & (d != OVERLAP)
        masks.append(valid.astype(ml_dtypes.bfloat16))
    return masks


def kernel(main, begin, end, in_proj_w, in_proj_b, out_proj_w, out_proj_b):
    import ml_dtypes
    from concourse.bass_utils import run_bass_kernel_spmd

    bf = ml_dtypes.bfloat16
    main = np.asarray(main, np.float32)
    begin = np.asarray(begin, np.float32)
    end = np.asarray(end, np.float32)
    in_proj_w = np.asarray(in_proj_w, np.float32)
    in_proj_b = np.asarray(in_proj_b, np.float32)
    out_proj_w = np.asarray(out_proj_w, np.float32)
    out_proj_b = np.asarray(out_proj_b, np.float32)

    D = EMBED_DIM
    scale = HEAD_DIM ** -0.5
    wq, wk, wv = in_proj_w[:D], in_proj_w[D:2 * D], in_proj_w[2 * D:]
    bq_, bv = in_proj_b[:D], in_proj_b[2 * D:3 * D]
    combined = np.concatenate([begin, main, end], axis=0)  # [N + 64, D]

    f8 = ml_dtypes.float8_e4m3
    wqT = np.ascontiguousarray(wq.T * scale).astype(bf)
    wkT = np.ascontiguousarray(wk.T).astype(bf)
    wv8 = (wv.T * 64.0).astype(f8)
    wo8T = np.ascontiguousarray(out_proj_w.T * 64.0).astype(f8)

    cst = np.zeros((128, NCONST), np.float32)
    cst[0:HEAD_DIM, 0:NUM_HEADS] = (bq_ * scale).reshape(NUM_HEADS, HEAD_DIM).T
    bo2 = out_proj_w @ bv + out_proj_b                      # [768]
    cst[:, 8:14] = bo2.reshape(KC, 128).T
    masks = _host_masks()
    mk = cst[:, 14:NCONST].view(bf)                         # [128, 384]
    mk[:, 0:128] = masks[0]
    mk[:, 128:320] = masks[1]
    mk[0:64, 320:384] = masks[2]

    shared = {"wqT": wqT, "wkT": wkT, "wo8T": wo8T, "cst": cst}
    in_maps = []
    for c in range(N_CORES):
        xTc = np.ascontiguousarray(combined[c * TOK: c * TOK + ROWS].T)
        xv8 = np.concatenate([xTc.astype(f8), wv8], axis=1)
        in_maps.append({**shared, "xT": xTc.astype(bf), "xv8T": xv8})

    nc = _get_program()
    res = run_bass_kernel_spmd(nc, in_maps, core_ids=list(range(N_CORES)),
                               **_program_cache.get("run_kwargs", {}))
    _program_cache["last_result"] = res

    outp = np.empty((N_LINES, 2 * D), np.float32)
    outp[:, :D] = main
    bo2 = out_proj_w @ bv + out_proj_b
    for c in range(N_CORES):
        # device output is x512-scaled (fp8 ctx x8, wo x64) without bias
        outp[c * TOK:(c + 1) * TOK, D:] = \
            res.results[c]["out"].T * (1.0 / 512) + bo2
    return outp


# revision 43
# speedup vs baseline: 1.2596x; 1.0038x over previous
"""Trainium2 Bass kernel for dilated sliding-window attention (AttnWrapper).

Reference computation (all fp32):
  combined = [begin | main | end]                       # [8256, 768]
  keys[t]  = combined[t + 32 + off], off in +-{4..32}   # 16 dilated window keys
  q = (main @ wq.T + bq) * 96**-0.5
  k/v = keys @ w{k,v}.T + b{k,v}
  attn = softmax(q.k), ctx = attn.v, out = [main | ctx @ wo.T + bo]

Sharding: tokens across 8 cores (1024 each) with a 64-row halo of the
combined buffer; weights replicated. Each core computes attn_outT
[768, 1024]; the host transposes and concatenates with main.

Device-side notes (v2 — all-bf16 matmul pipeline):
 - All matmul operands bf16 (cast on host). bf16 runs 1 cycle/col at any
   size on PE (f32r needs >=256 cols), so score matmuls stream only each
   key-chunk's valid token window (128/192/192/192/64) instead of 512.
 - bk dropped (softmax shift invariance); bv folded into bo2 = wo@bv + bo;
   q scale and bq folded into wq/bq on the host.
 - A ones-column per V head block makes the ctx matmul also produce the
   softmax denominator (row 96 of the ctx PSUM tile). 1/d runs on DVE
   (reciprocal_approx_fast — keeps ACT on the Exp table, avoiding
   ~1.3us table reloads per Ln/Exp switch), broadcast on GPSIMD,
   multiply on DVE; all off the PE critical path.
 - Phases: v-proj, k-proj, then q-proj software-pipelined with group-0
   attention (head h's attention is emitted during head h+1's q chains),
   then group-1 attention interleaved with group-0 out-projection.
 - Per-chunk input DMAs ordered so the first v matmul starts ~8us in.
"""

import numpy as np

EMBED_DIM = 768
NUM_HEADS = 8
HEAD_DIM = 96
OVERLAP = 32
HALO = 2 * OVERLAP          # 64 extra combined rows per core
N_LINES = 8192
N_CORES = 8
TOK = N_LINES // N_CORES    # 1024 tokens per core
ROWS = TOK + HALO           # 1088 combined rows per core
GRP = 512                   # tokens per attention group
NG = TOK // GRP             # 2 groups
# key chunks (start, end), valid token windows (w0, w1), mask index
CHUNKS = [(0, 128, 0, 128, 0), (128, 256, 64, 256, 1), (256, 384, 192, 384, 1),
          (384, 512, 320, 512, 1), (512, 576, 448, 512, 2)]
VBLK = HEAD_DIM + 1         # 97: v head block + ones column
KC = EMBED_DIM // 128       # 6 contraction chunks of 128
NVC = (ROWS + 127) // 128   # 9 v row-chunks (8x128 + 64)
NCONST = 8 + 6 + 192        # bq | bo2 | masks(bf16 pairs)


def _ctx_pieces():
    """Per-head splits of the 96 ctx rows into segments legal on both the
    per-head source side (base = row) and the packed-768 destination side
    (base = global row % 128), given the partition-access rule: base must
    be 0/32/64/96 and the range must not cross the next alignment level
    (0->128, 32->32, 64->64, 96->32 max sizes)."""
    def legal(s, size):
        return size <= {0: 128, 32: 32, 64: 64, 96: 32}.get(s, -1)

    out = []
    for h in range(NUM_HEADS):
        g0 = HEAD_DIM * h
        merged = []
        for a in range(g0, g0 + HEAD_DIM, 32):
            if merged:
                m0, m1 = merged[-1]
                if (m0 // 128 == (a + 31) // 128
                        and legal(m0 % 128, a + 32 - m0)
                        and legal(m0 - g0, a + 32 - m0)):
                    merged[-1] = (m0, a + 32)
                    continue
            merged.append((a, a + 32))
        out.append([(a - g0, b - g0, a // 256, (a // 128) % 2, a % 128)
                    for a, b in merged])
    return out


CTX_PIECES = _ctx_pieces()


def _build_program():
    import concourse.bacc as bacc
    import concourse.mybir as mybir
    from concourse.tile import TileContext

    f32 = mybir.dt.float32
    bf16 = mybir.dt.bfloat16
    D = EMBED_DIM

    nc = bacc.Bacc("TRN2", target_bir_lowering=False, debug=False,
                   enable_asserts=False, num_devices=N_CORES)

    fp8 = mybir.dt.float8e4
    PM = mybir.MatmulPerfMode.DoubleRow

    xT = nc.dram_tensor("xT", [D, ROWS], bf16, kind="ExternalInput")
    # x8 and wv8 side by side: one DMA per 256-row pair feeds the v matmul
    xv8T = nc.dram_tensor("xv8T", [D, ROWS + D], fp8, kind="ExternalInput")
    wqT = nc.dram_tensor("wqT", [D, D], bf16, kind="ExternalInput")
    wkT = nc.dram_tensor("wkT", [D, D], bf16, kind="ExternalInput")
    wo8T = nc.dram_tensor("wo8T", [D, D], fp8, kind="ExternalInput")
    cst = nc.dram_tensor("cst", [128, NCONST], f32, kind="ExternalInput")
    out = nc.dram_tensor("out", [D, TOK], f32, kind="ExternalOutput")

    with TileContext(nc) as tc:
        with tc.tile_pool(name="persist", bufs=1) as pers:
            vtile = [pers.tile([128, NUM_HEADS * VBLK], bf16, name=f"vt{r}")
                     for r in range(NVC)]
            for r in range(NVC):
                rows = min(128, ROWS - 128 * r)
                dst = vtile[r][0:rows, :].rearrange("p (b c) -> p b c", c=VBLK)
                nc.gpsimd.memset(dst[:, :, HEAD_DIM:VBLK], 1.0)

            # ---- input DMAs: per-chunk tiles, ordered by first use.
            # v runs first in fp8 DoubleRow: x|wv land as [128, 2, n] pairs.
            RD = ROWS + D
            xvp = [pers.tile([128, 2 * RD], fp8, name=f"xvp{q}")
                   for q in range(KC // 2)]
            xc = [pers.tile([128, ROWS], bf16, name=f"xc{c}")
                  for c in range(KC)]
            wkc = [pers.tile([128, D], bf16, name=f"wkc{c}")
                   for c in range(KC)]
            wqc = [pers.tile([128, D], bf16, name=f"wqc{c}")
                   for c in range(KC)]
            for q in range(KC // 2):
                nc.sync.dma_start(
                    xvp[q][:].rearrange("p (k r) -> p k r", k=2),
                    xv8T.ap()[256 * q:256 * (q + 1), :]
                    .rearrange("(k p) r -> p k r", p=128))
            for c in range(KC):
                nc.sync.dma_start(xc[c][:], xT.ap()[c * 128:(c + 1) * 128, :])
                nc.sync.dma_start(wkc[c][:], wkT.ap()[c * 128:(c + 1) * 128, :])
            cstt = pers.tile([128, NCONST], f32, name="cstt")
            nc.sync.dma_start(cstt[:], cst.ap())
            for c in range(KC):
                nc.sync.dma_start(wqc[c][:], wqT.ap()[c * 128:(c + 1) * 128, :])
            wop = [pers.tile([128, 2 * D], fp8, name=f"wop{t}")
                   for t in range(KC // 2)]
            for t in range(KC // 2):
                nc.sync.dma_start(
                    wop[t][:].rearrange("p (k n) -> p k n", k=2),
                    wo8T.ap()[256 * t:256 * (t + 1), :]
                    .rearrange("(k p) n -> p k n", p=128))

            bqt = cstt[0:HEAD_DIM, 0:NUM_HEADS]
            bo2t = cstt[:, 8:14]
            mkall = cstt[:, 14:NCONST].bitcast(bf16)   # [128, 384]
            masks = [mkall[:, 0:128], mkall[:, 128:320], mkall[0:64, 320:384]]

            qTh = [pers.tile([HEAD_DIM, TOK], bf16, name=f"qTh{h}")
                   for h in range(NUM_HEADS)]
            kTh = [pers.tile([HEAD_DIM, ROWS], bf16, name=f"kTh{h}")
                   for h in range(NUM_HEADS)]

            # ---- v projection (x-stationary, natural, fp8 DoubleRow)
            with tc.tile_pool(name="vpsum", bufs=2, space="PSUM") as vps:
                Copy = mybir.ActivationFunctionType.Copy
                for r in range(NVC):
                    rows = min(128, ROWS - 128 * r)
                    pv0 = vps.tile([128, 512], f32, tag="pv0", name="pv0")
                    pv1 = vps.tile([128, 256], f32, tag="pv1", name="pv1")
                    pv = [pv0, pv1]
                    for q in range(KC // 2):
                        xv = xvp[q][:].rearrange("p (k r) -> p k r", k=2)
                        for i, (nn, sz) in enumerate(((0, 512), (512, 256))):
                            nc.tensor.matmul(
                                pv[i][0:rows, 0:sz],
                                xv[:, :, 128 * r: 128 * r + rows],
                                xv[:, :, ROWS + nn: ROWS + nn + sz],
                                start=(q == 0), stop=(q == KC // 2 - 1),
                                perf_mode=PM)
                    # wv is host-scaled by 64 (e4m3 normal range);
                    # descale on the PSUM->SBUF copies
                    dst = vtile[r][0:rows, :].rearrange(
                        "p (b c) -> p b c", c=VBLK)
                    nc.scalar.activation(
                        dst[:, 0:5, 0:HEAD_DIM],
                        pv0[0:rows, 0:5 * HEAD_DIM]
                        .rearrange("p (b c) -> p b c", c=HEAD_DIM),
                        Copy, scale=1.0 / 64)
                    # head 5 straddles the 512 boundary: 480:512 | 0:64
                    nc.scalar.activation(dst[:, 5, 0:32],
                                         pv0[0:rows, 480:512],
                                         Copy, scale=1.0 / 64)
                    nc.scalar.activation(dst[:, 5, 32:HEAD_DIM],
                                         pv1[0:rows, 0:64],
                                         Copy, scale=1.0 / 64)
                    nc.scalar.activation(
                        dst[:, 6:8, 0:HEAD_DIM],
                        pv1[0:rows, 64:64 + 2 * HEAD_DIM]
                        .rearrange("p (b c) -> p b c", c=HEAD_DIM),
                        Copy, scale=1.0 / 64)

            # ---- per-head k/q chains + attention, then out-projection
            with tc.tile_pool(name="apool", bufs=2) as apool, \
                 tc.tile_pool(name="upool", bufs=1) as upool, \
                 tc.tile_pool(name="opool", bufs=4) as opool:
                # packed fp8 ctx: per group, 3 pair tiles covering the 768
                # concatenated head dims as [128, 2, 512]
                ctxP = [[upool.tile([128, 2 * GRP], fp8, name=f"ctxP{g}_{t}",
                                    tag=f"ctxP{g}_{t}")
                         for t in range(KC // 2)] for g in range(NG)]

                def k_head(h):
                    for n0, sz in ((0, 512), (512, 512), (1024, 64)):
                        ps = kqpsum.tile([HEAD_DIM, 512], f32, tag="pqk",
                                         name="ps_k")
                        for c in range(KC):
                            nc.tensor.matmul(
                                ps[:, 0:sz],
                                wkc[c][:, h * HEAD_DIM:(h + 1) * HEAD_DIM],
                                xc[c][:, n0: n0 + sz],
                                start=(c == 0), stop=(c == KC - 1))
                        nc.scalar.copy(kTh[h][:, n0:n0 + sz], ps[:, 0:sz])

                def q_head(h):
                    for n0 in (0, 512):
                        ps = kqpsum.tile([HEAD_DIM, 512], f32, tag="pqk",
                                         name="ps_q")
                        for c in range(KC):
                            nc.tensor.matmul(
                                ps[:],
                                wqc[c][:, h * HEAD_DIM:(h + 1) * HEAD_DIM],
                                xc[c][:, OVERLAP + n0: OVERLAP + n0 + 512],
                                start=(c == 0), stop=(c == KC - 1))
                        nc.vector.tensor_scalar_add(
                            qTh[h][:, n0:n0 + 512], ps[:], bqt[:, h:h + 1])

                def attn_score(g, h, c, st):
                    """Score matmul (windowed) + exp + mask for one chunk."""
                    (k0, k1, w0, w1, mi) = CHUNKS[c]
                    ksz = k1 - k0
                    win = w1 - w0
                    s_ps = spsum.tile([128, 192], f32, tag="s", name="s_ps")
                    nc.tensor.matmul(
                        s_ps[0:ksz, 0:win],
                        kTh[h][:, GRP * g + k0: GRP * g + k1],
                        qTh[h][:, GRP * g + w0: GRP * g + w1],
                        start=True, stop=True)
                    ex = apool.tile([128, 192], bf16, tag="ex", name="ex",
                                    bufs=8)
                    nc.scalar.activation(
                        ex[0:ksz, 0:win], s_ps[0:ksz, 0:win],
                        mybir.ActivationFunctionType.Exp)
                    nc.vector.tensor_tensor(
                        out=ex[0:ksz, 0:win], in0=ex[0:ksz, 0:win],
                        in1=masks[mi][0:ksz, 0:win],
                        op=mybir.AluOpType.mult)
                    st[(g, c)] = ex

                def attn_ctx(g, h, c, st, ctx_ps):
                    (k0, k1, w0, w1, mi) = CHUNKS[c]
                    ksz = k1 - k0
                    win = w1 - w0
                    nc.tensor.matmul(
                        ctx_ps[:, w0:w1],
                        vtile[4 * g + c][0:ksz, h * VBLK:(h + 1) * VBLK],
                        st[(g, c)][0:ksz, 0:win],
                        start=(c == 0), stop=(c == len(CHUNKS) - 1),
                        skip_group_check=True)

                def normalize(g, h, ctx_ps):
                    # 1/d on DVE (no ACT table switch), broadcast on GPSIMD.
                    # The d row goes through SBUF: the custom DVE op does
                    # not read PSUM at a partition offset correctly. The
                    # 1/8 folds a x8 ctx pre-scale for the fp8 cast.
                    rl0 = apool.tile([1, GRP], f32, tag="rl0", name="rl0",
                                     bufs=2)
                    nc.scalar.activation(rl0[:], ctx_ps[HEAD_DIM:VBLK, :],
                                         mybir.ActivationFunctionType.Copy,
                                         scale=0.125)
                    rl = apool.tile([1, GRP], f32, tag="rl", name="rl", bufs=2)
                    nc.vector.reciprocal_approx_fast(rl[:], rl0[:])
                    rdb = apool.tile([HEAD_DIM, GRP], f32, tag="rdb",
                                     name="rdb", bufs=2)
                    nc.gpsimd.partition_broadcast(rdb[:], rl[:])
                    for (s0, s1, t, kk, d0) in CTX_PIECES[h]:
                        dst = ctxP[g][t][:].rearrange("p (k n) -> p k n", k=2)
                        nc.vector.tensor_tensor(
                            out=dst[d0:d0 + s1 - s0, kk, :],
                            in0=ctx_ps[s0:s1, :], in1=rdb[s0:s1, :],
                            op=mybir.AluOpType.mult)

                def outproj_mm(i, dc, t, op):
                    nc.tensor.matmul(
                        op[:],
                        wop[t][:].rearrange("p (k n) -> p k n", k=2)
                        [:, :, dc * 128:(dc + 1) * 128],
                        ctxP[i][t][:].rearrange("p (k n) -> p k n", k=2),
                        start=(t == 0), stop=(t == KC // 2 - 1),
                        perf_mode=PM)

                def outproj_fin(i, dc, op, j):
                    # plain PSUM->SBUF copy (DMA cannot read PSUM),
                    # alternating ACT/DVE; the host folds the 1/512
                    # descale + bo2 bias into its transpose pass
                    ost = opool.tile([128, 512], f32, tag="ost", name="ost")
                    if j % 2 == 0:
                        nc.scalar.copy(ost[:], op[:])
                    else:
                        nc.vector.tensor_copy(ost[:], op[:])
                    nc.sync.dma_start(
                        out.ap()[dc * 128:(dc + 1) * 128,
                                 512 * i: 512 * (i + 1)], ost[:])

                def attention_both(a, ops=None):
                    """Both groups' attention for head a, chunk-interleaved
                    so every ctx matmul trails its score by ~2 PE slots
                    (exp+mask latency). `ops` optionally weaves ready
                    out-projection matmuls into the stream (last head)."""
                    st = {}
                    ctxps = {
                        0: apsum.tile([VBLK, GRP], f32, tag="ctx",
                                      name="ctx_ps"),
                        1: apsum.tile([VBLK, GRP], f32, tag="ctx",
                                      name="ctx_ps"),
                    }
                    stream = [("s", 0, 0), ("s", 0, 1), ("c", 0, 0),
                              ("s", 0, 2), ("c", 0, 1), ("s", 0, 3),
                              ("c", 0, 2), ("s", 0, 4), ("c", 0, 3),
                              ("s", 1, 0), ("c", 0, 4), ("n", 0, 0),
                              ("s", 1, 1), ("c", 1, 0), ("s", 1, 2),
                              ("c", 1, 1), ("s", 1, 3), ("c", 1, 2),
                              ("s", 1, 4), ("c", 1, 3), ("c", 1, 4),
                              ("n", 1, 0)]
                    for idx, (kind, g, c) in enumerate(stream):
                        if kind == "s":
                            attn_score(g, a, c, st)
                        elif kind == "c":
                            attn_ctx(g, a, c, st, ctxps[g])
                        else:
                            normalize(g, a, ctxps[g])
                        if ops and idx in ops:
                            for fn in ops[idx]:
                                fn()
                    return ctxps

                # head a's attention runs while head a+1's chains occupy
                # the PE; chains form unbroken MM runs (pstate-friendly)
                with tc.tile_pool(name="kqpsum", bufs=2,
                                  space="PSUM") as kqpsum, \
                     tc.tile_pool(name="apsum", bufs=2,
                                  space="PSUM") as apsum, \
                     tc.tile_pool(name="spsum", bufs=2,
                                  space="PSUM") as spsum, \
                     tc.tile_pool(name="op2sum", bufs=2,
                                  space="PSUM") as op2sum:
                    for it in range(NUM_HEADS):
                        k_head(it)
                        q_head(it)
                        if it > 0:
                            attention_both(it - 1)
                    # last head: weave group-0's out-projection into the
                    # group-1 half of the attention stream (group-0's
                    # normalize lands at stream index 11)
                    optile = {}

                    def u(dc, phase):
                        def f():
                            if phase == "a":
                                optile[dc] = op2sum.tile(
                                    [128, 512], f32, tag="po", name="ps_o")
                                outproj_mm(0, dc, 0, optile[dc])
                                outproj_mm(0, dc, 1, optile[dc])
                            else:
                                outproj_mm(0, dc, 2, optile[dc])
                                outproj_fin(0, dc, optile[dc], dc)
                        return f

                    ops = {12: [u(0, "a")], 14: [u(1, "a")],
                           16: [u(0, "b"), u(2, "a")],
                           18: [u(1, "b"), u(3, "a")],
                           20: [u(2, "b"), u(4, "a")],
                           21: [u(3, "b"), u(5, "a")]}
                    attention_both(NUM_HEADS - 1, ops)
                    u(4, "b")()
                    u(5, "b")()
                # attention PSUM freed: deep buffering for group 1
                with tc.tile_pool(name="opsum", bufs=6,
                                  space="PSUM") as opsum:
                    for dc in range(KC):
                        op = opsum.tile([128, 512], f32, tag="po",
                                        name="ps_o")
                        for t in range(KC // 2):
                            outproj_mm(1, dc, t, op)
                        outproj_fin(1, dc, op, dc)
    nc.compile()
    return nc


_program_cache = {}


def _get_program():
    if "nc" not in _program_cache:
        _program_cache["nc"] = _build_program()
    return _program_cache["nc"]


def _host_masks():
    # Three mask patterns: d = key - token offset within the chunk window.
    # m0 (first chunk): d = kk - mm; m1/m2 (later chunks): d = kk - mm + 64.
    import ml_dtypes
    masks = []
    for (nk, nw, off) in ((128, 128, 0), (128, 192, HALO), (64, 64, HALO)):
        kk, mm = np.meshgrid(np.arange(nk), np.arange(nw), indexing="ij")
        d = kk - mm + off
        valid = (d >= 0) & (d <= HALO) & (d % 4 == 0) & (d != OVERLAP)
        masks.append(valid.astype(ml_dtypes.bfloat16))
    return masks


def kernel(main, begin, end, in_proj_w, in_proj_b, out_proj_w, out_proj_b):
    import ml_dtypes
    from concourse.bass_utils import run_bass_kernel_spmd

    bf = ml_dtypes.bfloat16
    main = np.asarray(main, np.float32)
    begin = np.asarray(begin, np.float32)
    end = np.asarray(end, np.float32)
    in_proj_w = np.asarray(in_proj_w, np.float32)
    in_proj_b = np.asarray(in_proj_b, np.float32)
    out_proj_w = np.asarray(out_proj_w, np.float32)
    out_proj_b = np.asarray(out_proj_b, np.float32)

    D = EMBED_DIM
    scale = HEAD_DIM ** -0.5
    wq, wk, wv = in_proj_w[:D], in_proj_w[D:2 * D], in_proj_w[2 * D:]
    bq_, bv = in_proj_b[:D], in_proj_b[2 * D:3 * D]
    combined = np.concatenate([begin, main, end], axis=0)  # [N + 64, D]

    f8 = ml_dtypes.float8_e4m3
    wqT = np.ascontiguousarray(wq.T * scale).astype(bf)
    wkT = np.ascontiguousarray(wk.T).astype(bf)
    wv8 = (wv.T * 64.0).astype(f8)
    wo8T = np.ascontiguousarray(out_proj_w.T * 64.0).astype(f8)

    cst = np.zeros((128, NCONST), np.float32)
    cst[0:HEAD_DIM, 0:NUM_HEADS] = (bq_ * scale).reshape(NUM_HEADS, HEAD_DIM).T
    bo2 = out_proj_w @ bv + out_proj_b                      # [768]
    cst[:, 8:14] = bo2.reshape(KC, 128).T
    masks = _host_masks()
    mk = cst[:, 14:NCONST].view(bf)                         # [128, 384]
    mk[:, 0:128] = masks[0]
    mk[:, 128:320] = masks[1]
    mk[0:64, 320:384] = masks[2]

    shared = {"wqT": wqT, "wkT": wkT, "wo8T": wo8T, "cst": cst}
    in_maps = []
    for c in range(N_CORES):
        xTc = np.ascontiguousarray(combined[c * TOK: c * TOK + ROWS].T)
        xv8 = np.concatenate([xTc.astype(f8), wv8], axis=1)
        in_maps.append({**shared, "xT": xTc.astype(bf), "xv8T": xv8})

    nc = _get_program()
    res = run_bass_kernel_spmd(nc, in_maps, core_ids=list(range(N_CORES)),
                               **_program_cache.get("run_kwargs", {}))
    _program_cache["last_result"] = res

    outp = np.empty((N_LINES, 2 * D), np.float32)
    outp[:, :D] = main
    bo2 = out_proj_w @ bv + out_proj_b
    for c in range(N_CORES):
        # device output is x512-scaled (fp8 ctx x8, wo x64) without bias
        outp[c * TOK:(c + 1) * TOK, D:] = \
            res.results[c]["out"].T * (1.0 / 512) + bo2
    return outp


# revision 45
# speedup vs baseline: 1.2604x; 1.0006x over previous
"""Trainium2 Bass kernel for dilated sliding-window attention (AttnWrapper).

Reference computation (all fp32):
  combined = [begin | main | end]                       # [8256, 768]
  keys[t]  = combined[t + 32 + off], off in +-{4..32}   # 16 dilated window keys
  q = (main @ wq.T + bq) * 96**-0.5
  k/v = keys @ w{k,v}.T + b{k,v}
  attn = softmax(q.k), ctx = attn.v, out = [main | ctx @ wo.T + bo]

Sharding: tokens across 8 cores (1024 each) with a 64-row halo of the
combined buffer; weights replicated. Each core computes attn_outT
[768, 1024]; the host transposes and concatenates with main.

Device-side notes (v2 — all-bf16 matmul pipeline):
 - All matmul operands bf16 (cast on host). bf16 runs 1 cycle/col at any
   size on PE (f32r needs >=256 cols), so score matmuls stream only each
   key-chunk's valid token window (128/192/192/192/64) instead of 512.
 - bk dropped (softmax shift invariance); bv folded into bo2 = wo@bv + bo;
   q scale and bq folded into wq/bq on the host.
 - A ones-column per V head block makes the ctx matmul also produce the
   softmax denominator (row 96 of the ctx PSUM tile). 1/d runs on DVE
   (reciprocal_approx_fast — keeps ACT on the Exp table, avoiding
   ~1.3us table reloads per Ln/Exp switch), broadcast on GPSIMD,
   multiply on DVE; all off the PE critical path.
 - Phases: v-proj, k-proj, then q-proj software-pipelined with group-0
   attention (head h's attention is emitted during head h+1's q chains),
   then group-1 attention interleaved with group-0 out-projection.
 - Per-chunk input DMAs ordered so the first v matmul starts ~8us in.
"""

import numpy as np

EMBED_DIM = 768
NUM_HEADS = 8
HEAD_DIM = 96
OVERLAP = 32
HALO = 2 * OVERLAP          # 64 extra combined rows per core
N_LINES = 8192
N_CORES = 8
TOK = N_LINES // N_CORES    # 1024 tokens per core
ROWS = TOK + HALO           # 1088 combined rows per core
GRP = 512                   # tokens per attention group
NG = TOK // GRP             # 2 groups
# key chunks (start, end), valid token windows (w0, w1), mask index
CHUNKS = [(0, 128, 0, 128, 0), (128, 256, 64, 256, 1), (256, 384, 192, 384, 1),
          (384, 512, 320, 512, 1), (512, 576, 448, 512, 2)]
VBLK = HEAD_DIM + 1         # 97: v head block + ones column
KC = EMBED_DIM // 128       # 6 contraction chunks of 128
NVC = (ROWS + 127) // 128   # 9 v row-chunks (8x128 + 64)
NCONST = 8 + 6 + 192        # bq | bo2 | masks(bf16 pairs)


def _ctx_pieces():
    """Per-head splits of the 96 ctx rows into segments legal on both the
    per-head source side (base = row) and the packed-768 destination side
    (base = global row % 128), given the partition-access rule: base must
    be 0/32/64/96 and the range must not cross the next alignment level
    (0->128, 32->32, 64->64, 96->32 max sizes)."""
    def legal(s, size):
        return size <= {0: 128, 32: 32, 64: 64, 96: 32}.get(s, -1)

    out = []
    for h in range(NUM_HEADS):
        g0 = HEAD_DIM * h
        merged = []
        for a in range(g0, g0 + HEAD_DIM, 32):
            if merged:
                m0, m1 = merged[-1]
                if (m0 // 128 == (a + 31) // 128
                        and legal(m0 % 128, a + 32 - m0)
                        and legal(m0 - g0, a + 32 - m0)):
                    merged[-1] = (m0, a + 32)
                    continue
            merged.append((a, a + 32))
        out.append([(a - g0, b - g0, a // 256, (a // 128) % 2, a % 128)
                    for a, b in merged])
    return out


CTX_PIECES = _ctx_pieces()


def _build_program():
    import concourse.bacc as bacc
    import concourse.mybir as mybir
    from concourse.tile import TileContext

    f32 = mybir.dt.float32
    bf16 = mybir.dt.bfloat16
    D = EMBED_DIM

    nc = bacc.Bacc("TRN2", target_bir_lowering=False, debug=False,
                   enable_asserts=False, num_devices=N_CORES)

    fp8 = mybir.dt.float8e4
    PM = mybir.MatmulPerfMode.DoubleRow

    xT = nc.dram_tensor("xT", [D, ROWS], bf16, kind="ExternalInput")
    # x8 and wv8 side by side: one DMA per 256-row pair feeds the v matmul
    xv8T = nc.dram_tensor("xv8T", [D, ROWS + D], fp8, kind="ExternalInput")
    wqT = nc.dram_tensor("wqT", [D, D], bf16, kind="ExternalInput")
    wkT = nc.dram_tensor("wkT", [D, D], bf16, kind="ExternalInput")
    wo8T = nc.dram_tensor("wo8T", [D, D], fp8, kind="ExternalInput")
    cst = nc.dram_tensor("cst", [128, NCONST], f32, kind="ExternalInput")
    out = nc.dram_tensor("out", [D, TOK], f32, kind="ExternalOutput")

    with TileContext(nc) as tc:
        with tc.tile_pool(name="persist", bufs=1) as pers:
            vtile = [pers.tile([128, NUM_HEADS * VBLK], bf16, name=f"vt{r}")
                     for r in range(NVC)]
            for r in range(NVC):
                rows = min(128, ROWS - 128 * r)
                dst = vtile[r][0:rows, :].rearrange("p (b c) -> p b c", c=VBLK)
                nc.gpsimd.memset(dst[:, :, HEAD_DIM:VBLK], 1.0)

            # ---- input DMAs: per-chunk tiles, ordered by first use.
            # v runs first in fp8 DoubleRow: x|wv land as [128, 2, n] pairs.
            RD = ROWS + D
            xvp = [pers.tile([128, 2 * RD], fp8, name=f"xvp{q}")
                   for q in range(KC // 2)]
            xc = [pers.tile([128, ROWS], bf16, name=f"xc{c}")
                  for c in range(KC)]
            wkc = [pers.tile([128, D], bf16, name=f"wkc{c}")
                   for c in range(KC)]
            wqc = [pers.tile([128, D], bf16, name=f"wqc{c}")
                   for c in range(KC)]
            for q in range(KC // 2):
                xvv = xvp[q][:].rearrange("p (k r) -> p k r", k=2)
                src = xv8T.ap()[256 * q:256 * (q + 1), :] \
                    .rearrange("(k p) r -> p k r", p=128)
                nc.sync.dma_start(xvv[:, :, 0:ROWS], src[:, :, 0:ROWS])
                nc.sync.dma_start(xvv[:, :, ROWS:], src[:, :, ROWS:])
            for c in range(KC):
                nc.sync.dma_start(xc[c][:], xT.ap()[c * 128:(c + 1) * 128, :])
            for c in range(KC):
                nc.sync.dma_start(wkc[c][:], wkT.ap()[c * 128:(c + 1) * 128, :])
            cstt = pers.tile([128, NCONST], f32, name="cstt")
            nc.sync.dma_start(cstt[:], cst.ap())
            for c in range(KC):
                nc.sync.dma_start(wqc[c][:], wqT.ap()[c * 128:(c + 1) * 128, :])
            wop = [pers.tile([128, 2 * D], fp8, name=f"wop{t}")
                   for t in range(KC // 2)]
            for t in range(KC // 2):
                nc.sync.dma_start(
                    wop[t][:].rearrange("p (k n) -> p k n", k=2),
                    wo8T.ap()[256 * t:256 * (t + 1), :]
                    .rearrange("(k p) n -> p k n", p=128))

            bqt = cstt[0:HEAD_DIM, 0:NUM_HEADS]
            bo2t = cstt[:, 8:14]
            mkall = cstt[:, 14:NCONST].bitcast(bf16)   # [128, 384]
            masks = [mkall[:, 0:128], mkall[:, 128:320], mkall[0:64, 320:384]]

            qTh = [pers.tile([HEAD_DIM, TOK], bf16, name=f"qTh{h}")
                   for h in range(NUM_HEADS)]
            kTh = [pers.tile([HEAD_DIM, ROWS], bf16, name=f"kTh{h}")
                   for h in range(NUM_HEADS)]

            # ---- v projection (x-stationary, natural, fp8 DoubleRow)
            with tc.tile_pool(name="vpsum", bufs=3, space="PSUM") as vps:
                Copy = mybir.ActivationFunctionType.Copy
                for r in range(NVC):
                    rows = min(128, ROWS - 128 * r)
                    pv0 = vps.tile([128, 512], f32, tag="pv0", name="pv0")
                    pv1 = vps.tile([128, 256], f32, tag="pv1", name="pv1")
                    pv = [pv0, pv1]
                    for q in range(KC // 2):
                        xv = xvp[q][:].rearrange("p (k r) -> p k r", k=2)
                        for i, (nn, sz) in enumerate(((0, 512), (512, 256))):
                            nc.tensor.matmul(
                                pv[i][0:rows, 0:sz],
                                xv[:, :, 128 * r: 128 * r + rows],
                                xv[:, :, ROWS + nn: ROWS + nn + sz],
                                start=(q == 0), stop=(q == KC // 2 - 1),
                                perf_mode=PM)
                    # wv is host-scaled by 64 (e4m3 normal range);
                    # descale on the PSUM->SBUF copies
                    dst = vtile[r][0:rows, :].rearrange(
                        "p (b c) -> p b c", c=VBLK)
                    nc.scalar.activation(
                        dst[:, 0:5, 0:HEAD_DIM],
                        pv0[0:rows, 0:5 * HEAD_DIM]
                        .rearrange("p (b c) -> p b c", c=HEAD_DIM),
                        Copy, scale=1.0 / 64)
                    # head 5 straddles the 512 boundary: 480:512 | 0:64
                    nc.scalar.activation(dst[:, 5, 0:32],
                                         pv0[0:rows, 480:512],
                                         Copy, scale=1.0 / 64)
                    nc.scalar.activation(dst[:, 5, 32:HEAD_DIM],
                                         pv1[0:rows, 0:64],
                                         Copy, scale=1.0 / 64)
                    nc.scalar.activation(
                        dst[:, 6:8, 0:HEAD_DIM],
                        pv1[0:rows, 64:64 + 2 * HEAD_DIM]
                        .rearrange("p (b c) -> p b c", c=HEAD_DIM),
                        Copy, scale=1.0 / 64)

            # ---- per-head k/q chains + attention, then out-projection
            with tc.tile_pool(name="apool", bufs=2) as apool, \
                 tc.tile_pool(name="upool", bufs=1) as upool, \
                 tc.tile_pool(name="opool", bufs=4) as opool:
                # packed fp8 ctx: per group, 3 pair tiles covering the 768
                # concatenated head dims as [128, 2, 512]
                ctxP = [[upool.tile([128, 2 * GRP], fp8, name=f"ctxP{g}_{t}",
                                    tag=f"ctxP{g}_{t}")
                         for t in range(KC // 2)] for g in range(NG)]

                def k_head(h):
                    for n0, sz in ((0, 512), (512, 512), (1024, 64)):
                        ps = kqpsum.tile([HEAD_DIM, 512], f32, tag="pqk",
                                         name="ps_k")
                        for c in range(KC):
                            nc.tensor.matmul(
                                ps[:, 0:sz],
                                wkc[c][:, h * HEAD_DIM:(h + 1) * HEAD_DIM],
                                xc[c][:, n0: n0 + sz],
                                start=(c == 0), stop=(c == KC - 1))
                        nc.scalar.copy(kTh[h][:, n0:n0 + sz], ps[:, 0:sz])

                def q_head(h):
                    for n0 in (0, 512):
                        ps = kqpsum.tile([HEAD_DIM, 512], f32, tag="pqk",
                                         name="ps_q")
                        for c in range(KC):
                            nc.tensor.matmul(
                                ps[:],
                                wqc[c][:, h * HEAD_DIM:(h + 1) * HEAD_DIM],
                                xc[c][:, OVERLAP + n0: OVERLAP + n0 + 512],
                                start=(c == 0), stop=(c == KC - 1))
                        nc.vector.tensor_scalar_add(
                            qTh[h][:, n0:n0 + 512], ps[:], bqt[:, h:h + 1])

                def attn_score(g, h, c, st):
                    """Score matmul (windowed) + exp + mask for one chunk."""
                    (k0, k1, w0, w1, mi) = CHUNKS[c]
                    ksz = k1 - k0
                    win = w1 - w0
                    s_ps = spsum.tile([128, 192], f32, tag="s", name="s_ps")
                    nc.tensor.matmul(
                        s_ps[0:ksz, 0:win],
                        kTh[h][:, GRP * g + k0: GRP * g + k1],
                        qTh[h][:, GRP * g + w0: GRP * g + w1],
                        start=True, stop=True)
                    ex = apool.tile([128, 192], bf16, tag="ex", name="ex",
                                    bufs=8)
                    nc.scalar.activation(
                        ex[0:ksz, 0:win], s_ps[0:ksz, 0:win],
                        mybir.ActivationFunctionType.Exp)
                    nc.vector.tensor_tensor(
                        out=ex[0:ksz, 0:win], in0=ex[0:ksz, 0:win],
                        in1=masks[mi][0:ksz, 0:win],
                        op=mybir.AluOpType.mult)
                    st[(g, c)] = ex

                def attn_ctx(g, h, c, st, ctx_ps):
                    (k0, k1, w0, w1, mi) = CHUNKS[c]
                    ksz = k1 - k0
                    win = w1 - w0
                    nc.tensor.matmul(
                        ctx_ps[:, w0:w1],
                        vtile[4 * g + c][0:ksz, h * VBLK:(h + 1) * VBLK],
                        st[(g, c)][0:ksz, 0:win],
                        start=(c == 0), stop=(c == len(CHUNKS) - 1),
                        skip_group_check=True)

                def normalize(g, h, ctx_ps):
                    # 1/d on DVE (no ACT table switch), broadcast on GPSIMD.
                    # The d row goes through SBUF: the custom DVE op does
                    # not read PSUM at a partition offset correctly. The
                    # 1/8 folds a x8 ctx pre-scale for the fp8 cast.
                    rl0 = apool.tile([1, GRP], f32, tag="rl0", name="rl0",
                                     bufs=2)
                    nc.scalar.activation(rl0[:], ctx_ps[HEAD_DIM:VBLK, :],
                                         mybir.ActivationFunctionType.Copy,
                                         scale=0.125)
                    rl = apool.tile([1, GRP], f32, tag="rl", name="rl", bufs=2)
                    nc.vector.reciprocal_approx_fast(rl[:], rl0[:])
                    rdb = apool.tile([HEAD_DIM, GRP], f32, tag="rdb",
                                     name="rdb", bufs=2)
                    nc.gpsimd.partition_broadcast(rdb[:], rl[:])
                    for (s0, s1, t, kk, d0) in CTX_PIECES[h]:
                        dst = ctxP[g][t][:].rearrange("p (k n) -> p k n", k=2)
                        nc.vector.tensor_tensor(
                            out=dst[d0:d0 + s1 - s0, kk, :],
                            in0=ctx_ps[s0:s1, :], in1=rdb[s0:s1, :],
                            op=mybir.AluOpType.mult)

                def outproj_mm(i, dc, t, op):
                    nc.tensor.matmul(
                        op[:],
                        wop[t][:].rearrange("p (k n) -> p k n", k=2)
                        [:, :, dc * 128:(dc + 1) * 128],
                        ctxP[i][t][:].rearrange("p (k n) -> p k n", k=2),
                        start=(t == 0), stop=(t == KC // 2 - 1),
                        perf_mode=PM)

                def outproj_fin(i, dc, op, j):
                    # plain PSUM->SBUF copy (DMA cannot read PSUM),
                    # alternating ACT/DVE; the host folds the 1/512
                    # descale + bo2 bias into its transpose pass
                    ost = opool.tile([128, 512], f32, tag="ost", name="ost")
                    if j % 2 == 0:
                        nc.scalar.copy(ost[:], op[:])
                    else:
                        nc.vector.tensor_copy(ost[:], op[:])
                    nc.sync.dma_start(
                        out.ap()[dc * 128:(dc + 1) * 128,
                                 512 * i: 512 * (i + 1)], ost[:])

                def attention_both(a, ops=None):
                    """Both groups' attention for head a, chunk-interleaved
                    so every ctx matmul trails its score by ~2 PE slots
                    (exp+mask latency). `ops` optionally weaves ready
                    out-projection matmuls into the stream (last head)."""
                    st = {}
                    ctxps = {
                        0: apsum.tile([VBLK, GRP], f32, tag="ctx",
                                      name="ctx_ps"),
                        1: apsum.tile([VBLK, GRP], f32, tag="ctx",
                                      name="ctx_ps"),
                    }
                    stream = [("s", 0, 0), ("s", 0, 1), ("c", 0, 0),
                              ("s", 0, 2), ("c", 0, 1), ("s", 0, 3),
                              ("c", 0, 2), ("s", 0, 4), ("c", 0, 3),
                              ("s", 1, 0), ("c", 0, 4), ("n", 0, 0),
                              ("s", 1, 1), ("c", 1, 0), ("s", 1, 2),
                              ("c", 1, 1), ("s", 1, 3), ("c", 1, 2),
                              ("s", 1, 4), ("c", 1, 3), ("c", 1, 4),
                              ("n", 1, 0)]
                    for idx, (kind, g, c) in enumerate(stream):
                        if kind == "s":
                            attn_score(g, a, c, st)
                        elif kind == "c":
                            attn_ctx(g, a, c, st, ctxps[g])
                        else:
                            normalize(g, a, ctxps[g])
                        if ops and idx in ops:
                            for fn in ops[idx]:
                                fn()
                    return ctxps

                # head a's attention runs while head a+1's chains occupy
                # the PE; chains form unbroken MM runs (pstate-friendly)
                with tc.tile_pool(name="kqpsum", bufs=2,
                                  space="PSUM") as kqpsum, \
                     tc.tile_pool(name="apsum", bufs=2,
                                  space="PSUM") as apsum, \
                     tc.tile_pool(name="spsum", bufs=2,
                                  space="PSUM") as spsum, \
                     tc.tile_pool(name="op2sum", bufs=2,
                                  space="PSUM") as op2sum:
                    for it in range(NUM_HEADS):
                        k_head(it)
                        q_head(it)
                        if it > 0:
                            attention_both(it - 1)
                    # last head: weave group-0's out-projection into the
                    # group-1 half of the attention stream (group-0's
                    # normalize lands at stream index 11)
                    optile = {}

                    def u(dc, phase):
                        def f():
                            if phase == "a":
                                optile[dc] = op2sum.tile(
                                    [128, 512], f32, tag="po", name="ps_o")
                                outproj_mm(0, dc, 0, optile[dc])
                                outproj_mm(0, dc, 1, optile[dc])
                            else:
                                outproj_mm(0, dc, 2, optile[dc])
                                outproj_fin(0, dc, optile[dc], dc)
                        return f

                    ops = {12: [u(0, "a")], 14: [u(1, "a")],
                           16: [u(0, "b"), u(2, "a")],
                           18: [u(1, "b"), u(3, "a")],
                           20: [u(2, "b"), u(4, "a")],
                           21: [u(3, "b"), u(5, "a")]}
                    attention_both(NUM_HEADS - 1, ops)
                    u(4, "b")()
                    u(5, "b")()
                # attention PSUM freed: deep buffering for group 1
                with tc.tile_pool(name="opsum", bufs=6,
                                  space="PSUM") as opsum:
                    for dc in range(KC):
                        op = opsum.tile([128, 512], f32, tag="po",
                                        name="ps_o")
                        for t in range(KC // 2):
                            outproj_mm(1, dc, t, op)
                        outproj_fin(1, dc, op, dc)
    nc.compile()
    return nc


_program_cache = {}


def _get_program():
    if "nc" not in _program_cache:
        _program_cache["nc"] = _build_program()
    return _program_cache["nc"]


def _host_masks():
    # Three mask patterns: d = key - token offset within the chunk window.
    # m0 (first chunk): d = kk - mm; m1/m2 (later chunks): d = kk - mm + 64.
    import ml_dtypes
    masks = []
    for (nk, nw, off) in ((128, 128, 0), (128, 192, HALO), (64, 64, HALO)):
        kk, mm = np.meshgrid(np.arange(nk), np.arange(nw), indexing="ij")
        d = kk - mm + off
        valid = (d >= 0) & (d <= HALO) & (d % 4 == 0) & (d != OVERLAP)
        masks.append(valid.astype(ml_dtypes.bfloat16))
    return masks


def kernel(main, begin, end, in_proj_w, in_proj_b, out_proj_w, out_proj_b):
    import ml_dtypes
    from concourse.bass_utils import run_bass_kernel_spmd

    bf = ml_dtypes.bfloat16
    main = np.asarray(main, np.float32)
    begin = np.asarray(begin, np.float32)
    end = np.asarray(end, np.float32)
    in_proj_w = np.asarray(in_proj_w, np.float32)
    in_proj_b = np.asarray(in_proj_b, np.float32)
    out_proj_w = np.asarray(out_proj_w, np.float32)
    out_proj_b = np.asarray(out_proj_b, np.float32)

    D = EMBED_DIM
    scale = HEAD_DIM ** -0.5
    wq, wk, wv = in_proj_w[:D], in_proj_w[D:2 * D], in_proj_w[2 * D:]
    bq_, bv = in_proj_b[:D], in_proj_b[2 * D:3 * D]
    combined = np.concatenate([begin, main, end], axis=0)  # [N + 64, D]

    f8 = ml_dtypes.float8_e4m3
    wqT = np.ascontiguousarray(wq.T * scale).astype(bf)
    wkT = np.ascontiguousarray(wk.T).astype(bf)
    wv8 = (wv.T * 64.0).astype(f8)
    wo8T = np.ascontiguousarray(out_proj_w.T * 64.0).astype(f8)

    cst = np.zeros((128, NCONST), np.float32)
    cst[0:HEAD_DIM, 0:NUM_HEADS] = (bq_ * scale).reshape(NUM_HEADS, HEAD_DIM).T
    bo2 = out_proj_w @ bv + out_proj_b                      # [768]
    cst[:, 8:14] = bo2.reshape(KC, 128).T
    masks = _host_masks()
    mk = cst[:, 14:NCONST].view(bf)                         # [128, 384]
    mk[:, 0:128] = masks[0]
    mk[:, 128:320] = masks[1]
    mk[0:64, 320:384] = masks[2]

    shared = {"wqT": wqT, "wkT": wkT, "wo8T": wo8T, "cst": cst}
    in_maps = []
    for c in range(N_CORES):
        xTc = np.ascontiguousarray(combined[c * TOK: c * TOK + ROWS].T)
        xv8 = np.concatenate([xTc.astype(f8), wv8], axis=1)
        in_maps.append({**shared, "xT": xTc.astype(bf), "xv8T": xv8})

    nc = _get_program()
    res = run_bass_kernel_spmd(nc, in_maps, core_ids=list(range(N_CORES)),
                               **_program_cache.get("run_kwargs", {}))
    _program_cache["last_result"] = res

    outp = np.empty((N_LINES, 2 * D), np.float32)
    outp[:, :D] = main
    bo2 = out_proj_w @ bv + out_proj_b
    for c in range(N_CORES):
        # device output is x512-scaled (fp8 ctx x8, wo x64) without bias
        outp[c * TOK:(c + 1) * TOK, D:] = \
            res.results[c]["out"].T * (1.0 / 512) + bo2
    return outp
